# revision 1
# baseline (speedup 1.0000x reference)
"""EngineOrderFFT (Bluestein chirp-Z, fixed M=16384) Trainium2 kernel.

Strategy
--------
Pure data parallelism: batch dim B=64 is split across 8 NeuronCores
(8 samples/core). Each sample's variable-length DFT (length n_b) is computed
as a Bluestein transform with fixed FFT size M=16384 = 128*128, and each
16384-point (i)FFT is a two-stage Cooley-Tukey factorization executed as
128x128 fp16 matmuls on the tensor engine:

    n = n1 + 128*n2, k = k2 + 128*k1
    X[k2+128*k1] = sum_n1 D[n1,k1] * W[n1,k2] * sum_n2 a[n1+128*n2] * D[n2,k2]

Engine split per sample (8 channels batched in every instruction):
  sync   in/out DMAs
  gpsimd a-planes (x*chirp) + Fb plane replication across channel pages
  PE     4 matmul stages; twiddle/pointwise complex combines are absorbed
         into constant weights via PSUM accumulation (Karatsuba planes for
         the two twiddle layers, plain products for the Fa*Fb layer)
  ACT    PSUM -> fp16 SBUF evacuation after each stage
  DVE    twiddle/pointwise product planes (fp16 2x mode, replicated tables)

The final magnitude |conv[k]| equals |X[k]| (the output chirp has unit
modulus), so the kernel ships the complex conv planes (fp16, same bytes as
fp32 magnitudes) and the host takes hypot.

Host precompute (cheap, rpm-derived only): per-sample chirp tables
cos/-sin(pi*(t^2 mod 2n)/n), the FFT of the Bluestein kernel b (scaled
1/32), and the constant DFT/twiddle weight tables.
"""
import numpy as np

SF, RES, TS = 8192, 40, 1
B, L, C = 64, 8192, 8
M = 16384
NCORES = 8
SPC = B // NCORES  # samples per core

FBSCALE = 1.0 / 32.0
HSCALE = 1.0 / 16.0
KSCALE = 1.0 / 32.0  # HSCALE*KSCALE = (1/M) * (1/FBSCALE)

# ---------------------------------------------------------------------------
# constant tables (input-independent)
# ---------------------------------------------------------------------------


def _f16(x):
    return np.ascontiguousarray(x, dtype=np.float16)


def _rep8(x):
    return np.tile(x, (1, C))


def _build_const_tables():
    j = np.arange(128)
    D = np.exp(-2j * np.pi * np.outer(j, j) / 128.0)  # symmetric
    Dc = np.conj(D)
    Wt = np.exp(-2j * np.pi * np.outer(j, j) / M)  # fwd twiddle [n1,k2]
    W2 = np.conj(Wt)  # inv twiddle
    Dr, Di = D.real, D.imag
    Hr, Hi = (Dc * HSCALE).real, (Dc * HSCALE).imag
    Kr, Ki = (Dc * KSCALE).real[:, :64], (Dc * KSCALE).imag[:, :64]

    cols = []
    # F (fwd stage2, Karatsuba combine): F1,F2,F2n,F3,F4 [128,128]
    cols += [Dr + Di, Dr - Di, Di - Dr, -Di, Dr]
    # H (ifft stage1, Karatsuba): H_P=[Hr+Hi|Hi-Hr], H_Q=[Hr-Hi|Hi+Hr]
    cols += [np.concatenate([Hr + Hi, Hi - Hr], 1)]
    cols += [np.concatenate([Hr - Hi, Hi + Hr], 1)]
    # K (ifft stage2, Karatsuba): K1,K2,K2n,K3,K4 [128,64]
    cols += [Kr + Ki, Kr - Ki, Ki - Kr, -Ki, Kr]
    # twiddle tables replicated x8 channel pages (keeps DVE in 2x mode)
    cols += [_rep8(Wt.real), _rep8(-Wt.imag), _rep8(Wt.real + Wt.imag)]
    cols += [_rep8(W2.real), _rep8(-W2.imag), _rep8(W2.real + W2.imag)]
    cols += [np.concatenate([-Hi, Hr], 1)]  # H_R appended (2nd const DMA)
    ca = _f16(np.concatenate(cols, axis=1))

    cb = _f16(
        np.concatenate([Dr[:64], Di[:64], -Di[:64], Dr[:64]], axis=1)
    )  # [64, 512] = Dtab1|Dtab2
    return ca, cb


# column offsets in ca
_F = [0, 128, 256, 384, 512]  # F1,F2,F2n,F3,F4
_HA, _HB = 640, 896
_K = [1152, 1216, 1280, 1344, 1408]  # K1,K2,K2n,K3,K4
_WR, _WNI, _WS = 1472, 2496, 3520  # [128, 1024] each (replicated x8)
_W2R, _W2NI, _W2S = 4544, 5568, 6592
_HR = 7616
CA1_COLS = 4544
CA_COLS = 7872

_CONST_CACHE = {}


def _consts():
    if "ca" not in _CONST_CACHE:
        ca, cb = _build_const_tables()
        assert ca.shape[1] == CA_COLS, ca.shape
        _CONST_CACHE["ca"] = ca
        _CONST_CACHE["cb"] = cb
    return _CONST_CACHE["ca"], _CONST_CACHE["cb"]


# ---------------------------------------------------------------------------
# device module
# ---------------------------------------------------------------------------

_MODULE_CACHE = {}


def _build_module():
    import concourse.bass as bass
    from concourse import mybir

    dt = mybir.dt
    NB = 2  # per-sample buffer depth

    nc = bass.Bass("TRN2", target_bir_lowering=False, debug=False)

    xt = nc.dram_tensor("xt", [SPC, C, L], dt.float16, kind="ExternalInput").ap()
    # chirp tables, already replicated x8 channels: [SPC, 2, 64, 1024]
    ach = nc.dram_tensor("ach", [SPC, 2, 64, 1024], dt.float16, kind="ExternalInput").ap()
    # Fb planes (replicated x8 ch): [SPC, 3, 128, 1024] = (Fbr, +Fbi, -Fbi)*FBSCALE
    fbd = nc.dram_tensor("fbd", [SPC, 3, 128, 1024], dt.float16, kind="ExternalInput").ap()
    cad = nc.dram_tensor("cad", [128, CA_COLS], dt.float16, kind="ExternalInput").ap()
    cbd = nc.dram_tensor("cbd", [64, 512], dt.float16, kind="ExternalInput").ap()
    outr = nc.dram_tensor(
        "outr", [SPC, C, 2, L], dt.float16, kind="ExternalOutput"
    ).ap()

    ctx_list = []

    def sb(name, shape, dtype=None):
        t = nc.sbuf_tensor(name, shape, dtype or mybir.dt.float16)
        ap = t.__enter__()
        ctx_list.append(t)
        return ap

    def psum(name, shape):
        t = nc.psum_tensor(name, shape, mybir.dt.float32)
        ap = t.__enter__()
        ctx_list.append(t)
        return ap

    ca = sb("ca", [128, CA_COLS])
    cb = sb("cb", [64, 512])
    x_t = [sb(f"x{i}", [64, 1024]) for i in range(4)]
    ach_t = [sb(f"ach{i}", [64, 2048]) for i in range(4)]
    fbR = [sb(f"fbR{i}", [128, 3072]) for i in range(NB)]
    A_t = [sb(f"A{i}", [64, 2048]) for i in range(4)]
    Yf = [sb(f"Yf{i}", [128, 2048]) for i in range(NB)]
    Pb = [sb(f"Pb{i}", [128, 1024]) for i in range(NB)]
    Qnb = [sb(f"Qnb{i}", [128, 1024]) for i in range(NB)]
    Rb = [sb(f"Rb{i}", [128, 1024]) for i in range(NB)]
    Ff = [sb(f"Ff{i}", [128, 2048]) for i in range(NB)]
    CRb = [sb(f"CRb{i}", [128, 1024]) for i in range(NB)]
    CIb = [sb(f"CIb{i}", [128, 1024]) for i in range(NB)]
    Sf = [sb(f"Sf{i}", [128, 2048]) for i in range(NB)]
    P3b = [sb(f"P3b{i}", [128, 1024]) for i in range(NB)]
    Q3nb = [sb(f"Q3nb{i}", [128, 1024]) for i in range(NB)]
    R3b = [sb(f"R3b{i}", [128, 1024]) for i in range(NB)]
    ob = [sb(f"ob{i}", [64, 2048]) for i in range(NB)]
    # single-buffered DVE scratch (same-engine producer/consumer) -- but with
    # pair interleaving two samples' DVE groups are adjacent, so double them
    S1 = [sb(f"S1_{i}", [128, 1024]) for i in range(NB)]
    S3 = [sb(f"S3_{i}", [128, 1024]) for i in range(NB)]
    M1 = [sb(f"M1_{i}", [128, 1024]) for i in range(NB)]
    M2 = [sb(f"M2_{i}", [128, 1024]) for i in range(NB)]
    M3 = [sb(f"M3_{i}", [128, 1024]) for i in range(NB)]
    M4 = [sb(f"M4_{i}", [128, 1024]) for i in range(NB)]

    # two 4-bank psum regions; samples alternate regions by parity, and each
    # region runs its own strict phase sequence s1 -> s2 -> is1 -> is2
    psR = [psum("psR0", [128, 2048]), psum("psR1", [128, 2048])]

    csem = nc.alloc_semaphore("csem")
    c2sem = nc.alloc_semaphore("c2sem")
    cbsem = nc.alloc_semaphore("cbsem")
    smp = [nc.alloc_semaphore(f"smp{i}") for i in range(SPC)]
    osem = [nc.alloc_semaphore(f"osem{i}") for i in range(SPC)]
    fsem = [nc.alloc_semaphore(f"fsem{i}") for i in range(SPC)]
    vsem = nc.alloc_semaphore("vsem")
    psem = nc.alloc_semaphore("psem")
    ssem = nc.alloc_semaphore("ssem")
    gsem = nc.alloc_semaphore("gsem")

    # ---- emission orders (pair-interleaved) and semaphore target tables ----
    pairs = [(2 * p, 2 * p + 1) for p in range(SPC // 2)]

    pe_order = []   # (phase, s), phase in 0..3
    act_order = []  # (evac, s)
    dve_order = []  # (group, s), group in 0..2 (L1, CL, L3)
    gp_order = []   # (kind, s), kind 0=a-planes, 1=fbR
    for (sa, sb_) in pairs:
        for ph in range(4):
            pe_order += [(ph, sa), (ph, sb_)]
            act_order += [(ph, sa), (ph, sb_)]
        for g in range(3):
            dve_order += [(g, sa), (g, sb_)]
        gp_order += [(0, sa), (0, sb_)]
    gp_order = [e for e in gp_order if e[1] >= 2]

    PSEM = {}
    for i, key in enumerate(pe_order):
        PSEM[key] = i + 1
    SSEM = {}
    for i, key in enumerate(act_order):
        SSEM[key] = i + 1
    GSEM = {}
    g = 0
    for kind, s in gp_order:
        g += 2
        GSEM[(kind, s)] = g
    # DVE op positions per group (emission order below):
    #  L1/L3: P(+1), Qn(+2), S(+3), R(+4);  CL: M1,M2,CR,M3,M4,CI (+1..+6)
    VSEM = {}
    VOP = {}
    v = 4  # 4 startup a-plane ops on DVE (samples 0,1)
    for grp, s in dve_order:
        nops = (4, 4, 4)[grp]
        for k in range(1, nops + 1):
            VOP[(grp, s, k)] = v + k
        v += nops
        VSEM[(grp, s)] = v

    AluOp = mybir.AluOpType

    with nc.Block() as block:

        @block.sync
        def _(sync):
            def emit_in(s):
                b = s % NB
                b4 = s % 4
                if s >= 4:
                    if s - 4 < 2:
                        sync.wait_ge(vsem, 2 * (s - 4 + 1))
                    else:
                        sync.wait_ge(gsem, GSEM[(0, s - 4)])
                if s >= NB:
                    sync.wait_ge(vsem, VSEM[(1, s - NB)])
                sync.dma_start(
                    x_t[b4][:].rearrange("p (c n) -> p c n", c=C),
                    xt[s].rearrange("c (p n) -> p c n", n=128),
                ).then_inc(smp[s], 16)
                sync.dma_start(
                    ach_t[b4][:].rearrange("p (r n) -> p r n", r=2),
                    ach[s].rearrange("r p n -> p r n"),
                ).then_inc(smp[s], 16)
                sync.dma_start(
                    fbR[b][:].rearrange("p (f n) -> p f n", f=3),
                    fbd[s].rearrange("f p n -> p f n"),
                ).then_inc(fsem[s], 16)

            def emit_out(s):
                b = s % NB
                sync.wait_ge(ssem, SSEM[(3, s)])
                obv = ob[b][:].rearrange("p (q v) -> p q v", q=4)
                orv = outr[s].rearrange("(q j) r (p n) -> p q j r n", q=4, n=128)
                for r in range(2):
                    for jj in range(2):
                        sync.dma_start(
                            orv[:, :, jj, r, :],
                            obv[:, :, 256 * r + 128 * jj : 256 * r + 128 * jj + 128],
                        ).then_inc(osem[s], 16)

            # startup: tiny cb first, then sample-0 x/ach so Pool can start,
            # then the two halves of the big constant table between loads
            sync.dma_start(cb[:], cbd[:]).then_inc(cbsem, 16)
            sync.dma_start(
                x_t[0][:].rearrange("p (c n) -> p c n", c=C),
                xt[0].rearrange("c (p n) -> p c n", n=128),
            ).then_inc(smp[0], 16)
            sync.dma_start(
                ach_t[0][:].rearrange("p (r n) -> p r n", r=2),
                ach[0].rearrange("r p n -> p r n"),
            ).then_inc(smp[0], 16)
            sync.dma_start(ca[:, 0:CA1_COLS], cad[:, 0:CA1_COLS]).then_inc(csem, 16)
            sync.dma_start(
                fbR[0][:].rearrange("p (f n) -> p f n", f=3),
                fbd[0].rearrange("f p n -> p f n"),
            ).then_inc(fsem[0], 16)
            emit_in(1)
            sync.dma_start(ca[:, CA1_COLS:], cad[:, CA1_COLS:]).then_inc(c2sem, 16)
            emit_in(2)
            emit_in(3)
            for s in range(SPC):
                if s + 4 < SPC:
                    emit_in(s + 4)
                emit_out(s)

        @block.gpsimd
        def _(gp):
            for kind, s in gp_order:
                b4 = s % 4
                gp.wait_ge(smp[s], 32)
                if s >= 4:
                    gp.wait_ge(psem, PSEM[(0, s - 4)])  # A_t[b4] free
                nc.gpsimd.tensor_tensor(
                    A_t[b4][:, 0:1024], x_t[b4][:], ach_t[b4][:, 0:1024], AluOp.mult
                ).then_inc(gsem, 1)
                nc.gpsimd.tensor_tensor(
                    A_t[b4][:, 1024:2048],
                    x_t[b4][:],
                    ach_t[b4][:, 1024:2048],
                    AluOp.mult,
                ).then_inc(gsem, 1)

        @block.vector
        def _(vector):
            def chpages(ap):
                v_ = ap.rearrange("p (c u) -> p c u", c=C)
                return v_[:, :, 0:128], v_[:, :, 128:256]

            def prpages(ap):
                v_ = ap.rearrange("p (q u) -> p q u", q=4)
                return v_[:, :, 0:256], v_[:, :, 256:512]

            def flat4(ap):
                return ap.rearrange("p (q u) -> p q u", q=4)

            def flat8(ap):
                return ap.rearrange("p (c u) -> p c u", c=C)

            for s0 in (0, 1):
                vector.wait_ge(smp[s0], 32)
                nc.vector.tensor_tensor(
                    A_t[s0][:, 0:1024], x_t[s0][:], ach_t[s0][:, 0:1024], AluOp.mult
                ).then_inc(vsem, 1)
                nc.vector.tensor_tensor(
                    A_t[s0][:, 1024:2048],
                    x_t[s0][:],
                    ach_t[s0][:, 1024:2048],
                    AluOp.mult,
                ).then_inc(vsem, 1)
            first_dve = [True]
            for grp, s in dve_order:
                if first_dve[0]:
                    vector.wait_ge(csem, 16)
                    first_dve[0] = False
                    first_l3 = [True]
                b = s % NB
                if grp == 0:
                    # L1 (fwd twiddle, Karatsuba planes) from Yf
                    vector.wait_ge(ssem, SSEM[(0, s)])
                    if s >= NB:
                        vector.wait_ge(psem, PSEM[(1, s - NB)])  # Pb/Qnb/Rb free
                    yre, yim = chpages(Yf[b][:])
                    nc.vector.tensor_tensor(
                        flat8(Pb[b][:]), yre, flat8(ca[:, _WR : _WR + 1024]), AluOp.mult
                    ).then_inc(vsem, 1)
                    nc.vector.tensor_tensor(
                        flat8(Qnb[b][:]),
                        yim,
                        flat8(ca[:, _WNI : _WNI + 1024]),
                        AluOp.mult,
                    ).then_inc(vsem, 1)
                    nc.vector.tensor_tensor(
                        flat8(S1[b][:]), yre, yim, AluOp.add
                    ).then_inc(vsem, 1)
                    vector.wait_ge(vsem, VOP[(grp, s, 3)])  # S1 drained
                    nc.vector.tensor_tensor(
                        Rb[b][:], S1[b][:], ca[:, _WS : _WS + 1024], AluOp.mult
                    ).then_inc(vsem, 1)
                elif grp == 1:
                    # C-layer (Fa o Fb, Karatsuba planes) from Ff (pair-major)
                    vector.wait_ge(ssem, SSEM[(1, s)])
                    vector.wait_ge(fsem[s], 16)
                    if s >= NB:
                        vector.wait_ge(psem, PSEM[(2, s - NB)])  # plane bufs free
                    fre, fim = prpages(Ff[b][:])
                    nc.vector.tensor_tensor(
                        flat4(CRb[b][:]), fre, flat4(fbR[b][:, 0:1024]), AluOp.mult
                    ).then_inc(vsem, 1)  # P2 = Far*Fbr
                    nc.vector.tensor_tensor(
                        flat4(CIb[b][:]), fim, flat4(fbR[b][:, 1024:2048]), AluOp.mult
                    ).then_inc(vsem, 1)  # Q2n = Fai*(-Fbi)
                    nc.vector.tensor_tensor(
                        flat4(M1[b][:]), fre, fim, AluOp.add
                    ).then_inc(vsem, 1)  # s2 = Far+Fai
                    vector.wait_ge(vsem, VOP[(grp, s, 3)])  # s2 drained
                    nc.vector.tensor_tensor(
                        M2[b][:], M1[b][:], fbR[b][:, 2048:3072], AluOp.mult
                    ).then_inc(vsem, 1)  # R2 = s2*(Fbr+Fbi)
                else:
                    # L3 (inv twiddle, Karatsuba planes) from Sf
                    if first_l3[0]:
                        vector.wait_ge(c2sem, 16)
                        first_l3[0] = False
                    vector.wait_ge(ssem, SSEM[(2, s)])
                    if s >= NB:
                        vector.wait_ge(psem, PSEM[(3, s - NB)])  # P3b/.. free
                    sre, sim_ = chpages(Sf[b][:])
                    nc.vector.tensor_tensor(
                        flat8(P3b[b][:]),
                        sre,
                        flat8(ca[:, _W2R : _W2R + 1024]),
                        AluOp.mult,
                    ).then_inc(vsem, 1)
                    nc.vector.tensor_tensor(
                        flat8(Q3nb[b][:]),
                        sim_,
                        flat8(ca[:, _W2NI : _W2NI + 1024]),
                        AluOp.mult,
                    ).then_inc(vsem, 1)
                    nc.vector.tensor_tensor(
                        flat8(S3[b][:]), sre, sim_, AluOp.add
                    ).then_inc(vsem, 1)
                    vector.wait_ge(vsem, VOP[(grp, s, 3)])  # S3 drained
                    nc.vector.tensor_tensor(
                        R3b[b][:], S3[b][:], ca[:, _W2S : _W2S + 1024], AluOp.mult
                    ).then_inc(vsem, 1)

        @block.tensor
        def _(tensor):
            mm = nc.tensor.matmul
            first_pe = [True]
            first_is1 = [True]

            def phase_s1(s):
                b = s % NB
                ps = psR[s % 2]
                if first_pe[0]:
                    tensor.wait_ge(cbsem, 16)  # cb loaded
                    first_pe[0] = False
                if s < 2:
                    tensor.wait_ge(vsem, 2 * (s + 1))  # startup a-planes on DVE
                else:
                    tensor.wait_ge(gsem, GSEM[(0, s)])
                if s >= NB:
                    tensor.wait_ge(ssem, SSEM[(3, s - NB)])  # region free
                b4 = s % 4
                for c in range(C):
                    o = ps[:, 256 * c : 256 * c + 256]
                    mm(
                        o,
                        A_t[b4][:, 128 * c : 128 * c + 128],
                        cb[:, 0:256],
                        start=True,
                        stop=False,
                    )
                    i = mm(
                        o,
                        A_t[b4][:, 1024 + 128 * c : 1024 + 128 * c + 128],
                        cb[:, 256:512],
                        start=False,
                        stop=True,
                    )
                    if c == C - 1:
                        i.then_inc(psem, 1)

            def phase_s2(s):
                b = s % NB
                ps = psR[s % 2]
                tensor.wait_ge(vsem, VOP[(0, s, 1)])  # Pb ready
                tensor.wait_ge(csem, 16)  # ca loaded
                tensor.wait_ge(ssem, SSEM[(0, s)])  # region free after evacY
                srcs = [
                    (Pb[b], _F[0], 0, True, False, None),
                    (Qnb[b], _F[0], 256, False, False, VOP[(0, s, 2)]),
                    (Qnb[b], _F[1], 0, False, False, None),
                    (Pb[b], _F[2], 256, False, False, None),
                    (Rb[b], _F[3], 0, False, False, VOP[(0, s, 4)]),
                    (Rb[b], _F[4], 256, False, True, None),
                ]
                for wi, (buf, fofs, oofs, st, sp, wv) in enumerate(srcs):
                    if wv is not None:
                        tensor.wait_ge(vsem, wv)
                    for q in range(4):
                        i = mm(
                            ps[:, 512 * q + oofs : 512 * q + oofs + 256],
                            ca[:, fofs : fofs + 128],
                            buf[:, 256 * q : 256 * q + 256],
                            start=st,
                            stop=sp,
                        )
                        if wi == 5 and q == 3:
                            i.then_inc(psem, 1)

            def phase_is1(s):
                b = s % NB
                ps = psR[s % 2]
                if first_is1[0]:
                    tensor.wait_ge(c2sem, 16)  # H_R lives in the 2nd const DMA
                    first_is1[0] = False
                tensor.wait_ge(vsem, VOP[(1, s, 1)])  # P2 ready
                tensor.wait_ge(ssem, SSEM[(1, s)])
                for c in range(C):
                    # even channel opens its bank; odd writes the other half
                    mm(
                        ps[:, 256 * c : 256 * c + 256],
                        CRb[b][:, 128 * c : 128 * c + 128],
                        ca[:, _HA : _HA + 256],
                        start=(c % 2 == 0),
                        stop=False,
                    )
                tensor.wait_ge(vsem, VOP[(1, s, 2)])  # Q2n ready
                for c in range(C):
                    mm(
                        ps[:, 256 * c : 256 * c + 256],
                        CIb[b][:, 128 * c : 128 * c + 128],
                        ca[:, _HB : _HB + 256],
                        start=False,
                        stop=False,
                    )
                tensor.wait_ge(vsem, VOP[(1, s, 4)])  # R2 ready
                for c in range(C):
                    i = mm(
                        ps[:, 256 * c : 256 * c + 256],
                        M2[b][:, 128 * c : 128 * c + 128],
                        ca[:, _HR : _HR + 256],
                        start=False,
                        stop=(c % 2 == 1),
                    )
                    if c == C - 1:
                        i.then_inc(psem, 1)

            def phase_is2(s):
                b = s % NB
                ps = psR[s % 2]
                tensor.wait_ge(vsem, VOP[(2, s, 1)])  # P3b ready
                tensor.wait_ge(ssem, SSEM[(2, s)])
                srcs = [
                    (P3b[b], _K[0], 0, True, False, None),
                    (Q3nb[b], _K[0], 256, False, False, VOP[(2, s, 2)]),
                    (Q3nb[b], _K[1], 0, False, False, None),
                    (P3b[b], _K[2], 256, False, False, None),
                    (R3b[b], _K[3], 0, False, False, VOP[(2, s, 4)]),
                    (R3b[b], _K[4], 256, False, True, None),
                ]
                for wi, (buf, kofs, oofs, st, sp, wv) in enumerate(srcs):
                    if wv is not None:
                        tensor.wait_ge(vsem, wv)
                    for q in range(4):
                        i = mm(
                            ps[0:64, 512 * q + oofs : 512 * q + oofs + 256],
                            ca[:, kofs : kofs + 64],
                            buf[:, 256 * q : 256 * q + 256],
                            start=st,
                            stop=sp,
                        )
                        if wi == 5 and q == 3:
                            i.then_inc(psem, 1)

            phase_fns = [phase_s1, phase_s2, phase_is1, phase_is2]
            for ph, s in pe_order:
                phase_fns[ph](s)

        @block.scalar
        def _(scalar):
            for ph, s in act_order:
                b = s % NB
                ps = psR[s % 2]
                scalar.wait_ge(psem, PSEM[(ph, s)])
                if ph == 0:
                    nc.scalar.copy(Yf[b][:], ps[:, 0:2048]).then_inc(ssem, 1)
                elif ph == 1:
                    nc.scalar.copy(Ff[b][:], ps[:, 0:2048]).then_inc(ssem, 1)
                elif ph == 2:
                    nc.scalar.copy(Sf[b][:], ps[:, 0:2048]).then_inc(ssem, 1)
                else:
                    if s >= NB:
                        scalar.wait_ge(osem[s - NB], 64)
                    nc.scalar.copy(ob[b][:], ps[0:64, 0:2048]).then_inc(ssem, 1)

    for t in reversed(ctx_list):
        t.__exit__(None, None, None)

    return nc


def _get_module():
    if "nc" not in _MODULE_CACHE:
        _MODULE_CACHE["nc"] = _build_module()
    return _MODULE_CACHE["nc"]


# ---------------------------------------------------------------------------
# host side
# ---------------------------------------------------------------------------


def _host_tables(rpm):
    """Per-sample chirp tables (replicated x8 channels) + Fb planes."""
    pad = np.floor((RES * 60.0 / rpm.astype(np.float64) - TS) * SF).astype(np.int64)
    n_arr = L + pad
    t = np.arange(L, dtype=np.int64)
    m = np.arange(M, dtype=np.int64)
    mm = np.minimum(m, M - m)

    ach = np.empty((B, 2, 64, 1024), np.float16)
    fbp = np.empty((B, 3, 128, 1024), np.float16)
    for b in range(B):
        n = int(n_arr[b])
        two_n = 2 * n
        ph = np.pi * ((t * t) % two_n) / n
        cosg = np.cos(ph).astype(np.float16).reshape(64, 128)
        nsing = (-np.sin(ph)).astype(np.float16).reshape(64, 128)
        ach[b, 0] = np.tile(cosg, (1, C))
        ach[b, 1] = np.tile(nsing, (1, C))
        phb = np.pi * ((mm * mm) % two_n) / n
        Fb = np.fft.fft(np.exp(1j * phb)).reshape(128, 128) * FBSCALE
        fbp[b, 0] = np.tile(Fb.real.astype(np.float16), (1, C))
        fbp[b, 1] = np.tile((-Fb.imag).astype(np.float16), (1, C))
        fbp[b, 2] = np.tile((Fb.real + Fb.imag).astype(np.float16), (1, C))
    return ach, fbp


LAST_EXEC_WALL_NS = [None]


def kernel(inputs, rpm):
    inputs = np.ascontiguousarray(inputs, dtype=np.float32)  # [B, L, C]
    rpm = np.ascontiguousarray(rpm, dtype=np.float32)

    ca, cb = _consts()
    ach, fbp = _host_tables(rpm)
    xt = np.ascontiguousarray(inputs.transpose(0, 2, 1)).astype(np.float16)

    nc = _get_module()
    in_maps = []
    for g in range(NCORES):
        s0 = g * SPC
        in_maps.append(
            {
                "xt": xt[s0 : s0 + SPC],
                "ach": ach[s0 : s0 + SPC],
                "fbd": fbp[s0 : s0 + SPC],
                "cad": ca,
                "cbd": cb,
            }
        )

    import time

    from concourse.bass_utils import run_bass_kernel_spmd

    t0 = time.perf_counter_ns()
    res = run_bass_kernel_spmd(nc, in_maps, list(range(NCORES)))
    LAST_EXEC_WALL_NS[0] = time.perf_counter_ns() - t0

    out = np.empty((B, L, C), np.float32)
    for g in range(NCORES):
        planes = np.asarray(res.results[g]["outr"], np.float32)  # [SPC, C, 2, L]
        mag = np.hypot(planes[:, :, 0, :], planes[:, :, 1, :])  # [SPC, C, L]
        out[g * SPC : (g + 1) * SPC] = mag.transpose(0, 2, 1)
    return out



# revision 3
# speedup vs baseline: 1.1048x; 1.1048x over previous
"""EngineOrderFFT (Bluestein chirp-Z, fixed M=16384) Trainium2 kernel, V2.

Strategy
--------
Pure data parallelism: batch dim B=64 split across 8 NeuronCores
(8 samples/core). Each sample's variable-length DFT (length n_b) is a
Bluestein transform with fixed FFT size M=16384 = 128*128; each
16384-point (i)FFT is a two-stage Cooley-Tukey factorization executed as
fp16 matmuls on the tensor engine.

V2 changes vs V1:
  * is2 (inverse stage 2) runs in data-as-lhsT form: per channel the
    twiddled planes P3/Q3n/R3 [k2,m1] are the stationary operand and the
    Karatsuba combine tables K_P/K_Q/K_R [k2, m2(re|im)] are the moving
    operand -> 3072 output columns/sample instead of 6144, and the output
    lands as [m1, ch*(re64|im64)] = [128, 1024] (half the PSUM, cheaper
    evacuation).
  * All chirp/twiddle/Fb tables are stored un-replicated; the elementwise
    products broadcast them across the 8 channel pages with 0-stride APs
    (same DVE cost, 8x less DMA + SBUF).

Engine split per sample (8 channels batched in every instruction):
  sync   in/out DMAs
  gpsimd a-planes (x*chirp)
  PE     4 matmul stages (s1 16mm/4096col, s2 24mm/6144, is1 24mm/6144,
         is2 24mm/3072)
  ACT    PSUM -> fp16 SBUF evacuation after each stage
  DVE    twiddle/pointwise Karatsuba product planes (fp16 2x mode)

|conv[k]| is computed on the host from the shipped re/im planes.
"""
import numpy as np

SF, RES, TS = 8192, 40, 1
B, L, C = 64, 8192, 8
M = 16384
NCORES = 8
SPC = B // NCORES  # samples per core

FBSCALE = 1.0 / 32.0
HSCALE = 1.0 / 16.0
KSCALE = 1.0 / 32.0  # HSCALE*KSCALE = (1/M) * (1/FBSCALE)

# ---------------------------------------------------------------------------
# constant tables (input-independent)
# ---------------------------------------------------------------------------


def _f16(x):
    return np.ascontiguousarray(x, dtype=np.float16)


def _build_const_tables():
    j = np.arange(128)
    D = np.exp(-2j * np.pi * np.outer(j, j) / 128.0)  # symmetric
    Dc = np.conj(D)
    Wt = np.exp(-2j * np.pi * np.outer(j, j) / M)  # fwd twiddle [n1,k2]
    W2 = np.conj(Wt)  # inv twiddle
    Dr, Di = D.real, D.imag
    Hr, Hi = (Dc * HSCALE).real, (Dc * HSCALE).imag
    Kr, Ki = (Dc * KSCALE).real[:, :64], (Dc * KSCALE).imag[:, :64]

    cols = []
    # chunk 1 (cols 0:1024): W twiddle (L1), F tables (s2)
    cols += [Wt.real, -Wt.imag, Wt.real + Wt.imag]  # _WR,_WNI,_WS [128,128]
    cols += [Dr + Di, Dr - Di, Di - Dr, -Di, Dr]  # F1,F2,F2n,F3,F4 [128,128]
    # chunk 2 (cols 1024:2560): H (is1), W2 (L3), K (is2)
    cols += [np.concatenate([Hr + Hi, Hi - Hr], 1)]  # H_P [128,256]
    cols += [np.concatenate([Hr - Hi, Hi + Hr], 1)]  # H_Q
    cols += [np.concatenate([-Hi, Hr], 1)]  # H_R
    cols += [W2.real, -W2.imag, W2.real + W2.imag]  # _W2R,_W2NI,_W2S
    cols += [np.concatenate([Kr + Ki, Ki - Kr], 1)]  # K_P [128,128]
    cols += [np.concatenate([Kr - Ki, Ki + Kr], 1)]  # K_Q
    cols += [np.concatenate([-Ki, Kr], 1)]  # K_R
    ca = _f16(np.concatenate(cols, axis=1))

    cb = _f16(
        np.concatenate([Dr[:64], Di[:64], -Di[:64], Dr[:64]], axis=1)
    )  # [64, 512] = Dtab1|Dtab2
    return ca, cb


# column offsets in ca
_WR, _WNI, _WS = 0, 128, 256
_F = [384, 512, 640, 768, 896]  # F1,F2,F2n,F3,F4
_HA, _HB, _HR = 1024, 1280, 1536
_W2R, _W2NI, _W2S = 1792, 1920, 2048
_KP, _KQ, _KR = 2176, 2304, 2432
CA1_COLS = 1024
CA_COLS = 2560

_CONST_CACHE = {}


def _consts():
    if "ca" not in _CONST_CACHE:
        ca, cb = _build_const_tables()
        assert ca.shape[1] == CA_COLS, ca.shape
        _CONST_CACHE["ca"] = ca
        _CONST_CACHE["cb"] = cb
    return _CONST_CACHE["ca"], _CONST_CACHE["cb"]


# ---------------------------------------------------------------------------
# device module
# ---------------------------------------------------------------------------

_MODULE_CACHE = {}


def _build_module():
    import concourse.bass as bass
    from concourse import mybir

    dt = mybir.dt
    NB = 2  # per-sample buffer depth

    nc = bass.Bass("TRN2", target_bir_lowering=False, debug=False)

    xt = nc.dram_tensor("xt", [SPC, C, L], dt.float16, kind="ExternalInput").ap()
    # chirp tables (un-replicated): [SPC, 2, 64, 128] = (cos, -sin)
    ach = nc.dram_tensor("ach", [SPC, 2, 64, 128], dt.float16, kind="ExternalInput").ap()
    # Fb planes (un-replicated): [SPC, 3, 128, 128] = (Fbr, -Fbi, Fbr+Fbi)*FBSCALE
    fbd = nc.dram_tensor("fbd", [SPC, 3, 128, 128], dt.float16, kind="ExternalInput").ap()
    cad = nc.dram_tensor("cad", [128, CA_COLS], dt.float16, kind="ExternalInput").ap()
    cbd = nc.dram_tensor("cbd", [64, 512], dt.float16, kind="ExternalInput").ap()
    # out: [SPC, 128(m1), 8ch * (re64|im64)(m2)] ; conv[m1+128*m2]
    outr = nc.dram_tensor("outr", [SPC, 128, 1024], dt.float16, kind="ExternalOutput").ap()

    ctx_list = []

    def sb(name, shape, dtype=None):
        t = nc.sbuf_tensor(name, shape, dtype or mybir.dt.float16)
        ap = t.__enter__()
        ctx_list.append(t)
        return ap

    def psum(name, shape):
        t = nc.psum_tensor(name, shape, mybir.dt.float32)
        ap = t.__enter__()
        ctx_list.append(t)
        return ap

    ca = sb("ca", [128, CA_COLS])
    cb = sb("cb", [64, 512])
    x_t = [sb(f"x{i}", [64, 1024]) for i in range(4)]
    ach_t = [sb(f"ach{i}", [64, 256]) for i in range(4)]
    fbR = [sb(f"fbR{i}", [128, 384]) for i in range(NB)]
    A_t = [sb(f"A{i}", [64, 2048]) for i in range(4)]
    Yf = [sb(f"Yf{i}", [128, 2048]) for i in range(NB)]
    Pb = [sb(f"Pb{i}", [128, 1024]) for i in range(NB)]
    Qnb = [sb(f"Qnb{i}", [128, 1024]) for i in range(NB)]
    Rb = [sb(f"Rb{i}", [128, 1024]) for i in range(NB)]
    Ff = [sb(f"Ff{i}", [128, 2048]) for i in range(NB)]
    CRb = [sb(f"CRb{i}", [128, 1024]) for i in range(NB)]
    CIb = [sb(f"CIb{i}", [128, 1024]) for i in range(NB)]
    Sf = [sb(f"Sf{i}", [128, 2048]) for i in range(NB)]
    P3b = [sb(f"P3b{i}", [128, 1024]) for i in range(NB)]
    Q3nb = [sb(f"Q3nb{i}", [128, 1024]) for i in range(NB)]
    R3b = [sb(f"R3b{i}", [128, 1024]) for i in range(NB)]
    ob = [sb(f"ob{i}", [128, 1024]) for i in range(NB)]
    S1 = [sb(f"S1_{i}", [128, 1024]) for i in range(NB)]
    S3 = [sb(f"S3_{i}", [128, 1024]) for i in range(NB)]
    M1 = [sb(f"M1_{i}", [128, 1024]) for i in range(NB)]
    M2 = [sb(f"M2_{i}", [128, 1024]) for i in range(NB)]

    # two 4-bank psum regions; samples alternate regions by parity
    psR = [psum("psR0", [128, 2048]), psum("psR1", [128, 2048])]

    csem = nc.alloc_semaphore("csem")
    c2sem = nc.alloc_semaphore("c2sem")
    cbsem = nc.alloc_semaphore("cbsem")
    smp = [nc.alloc_semaphore(f"smp{i}") for i in range(SPC)]
    osem = [nc.alloc_semaphore(f"osem{i}") for i in range(SPC)]
    fsem = [nc.alloc_semaphore(f"fsem{i}") for i in range(SPC)]
    vsem = nc.alloc_semaphore("vsem")
    psem = nc.alloc_semaphore("psem")
    ssem = nc.alloc_semaphore("ssem")
    gsem = nc.alloc_semaphore("gsem")

    # ---- emission orders (pair-interleaved) and semaphore target tables ----
    pairs = [(2 * p, 2 * p + 1) for p in range(SPC // 2)]

    pe_order = []   # (phase, s), phase in 0..3
    act_order = []  # (evac, s)
    dve_order = []  # (group, s), group in 0..2 (L1, CL, L3)
    gp_order = []   # (kind, s), kind 0=a-planes
    for (sa, sb_) in pairs:
        for ph in range(4):
            pe_order += [(ph, sa), (ph, sb_)]
            act_order += [(ph, sa), (ph, sb_)]
        for g in range(3):
            dve_order += [(g, sa), (g, sb_)]
        gp_order += [(0, sa), (0, sb_)]
    gp_order = [e for e in gp_order if e[1] >= 2]

    PSEM = {}
    for i, key in enumerate(pe_order):
        PSEM[key] = i + 1
    SSEM = {}
    for i, key in enumerate(act_order):
        SSEM[key] = i + 1
    GSEM = {}
    g = 0
    for kind, s in gp_order:
        g += 2
        GSEM[(kind, s)] = g
    # DVE op positions per group: P(+1), Qn(+2), S(+3), R(+4)
    VSEM = {}
    VOP = {}
    v = 4  # 4 startup a-plane ops on DVE (samples 0,1)
    for grp, s in dve_order:
        for k in range(1, 5):
            VOP[(grp, s, k)] = v + k
        v += 4
        VSEM[(grp, s)] = v

    AluOp = mybir.AluOpType

    def bcast8(tab):
        """[P,128] table -> [P, 8, 128] zero-stride channel broadcast."""
        p = tab.shape[0]
        return tab.rearrange("p (o u) -> p o u", o=1).broadcast_to((p, 8, 128))

    with nc.Block() as block:

        @block.sync
        def _(sync):
            def emit_in(s):
                b = s % NB
                b4 = s % 4
                if s >= 4:
                    if s - 4 < 2:
                        sync.wait_ge(vsem, 2 * (s - 4 + 1))
                    else:
                        sync.wait_ge(gsem, GSEM[(0, s - 4)])
                if s >= NB:
                    sync.wait_ge(vsem, VSEM[(1, s - NB)])
                sync.dma_start(
                    x_t[b4][:].rearrange("p (c n) -> p c n", c=C),
                    xt[s].rearrange("c (p n) -> p c n", n=128),
                ).then_inc(smp[s], 16)
                sync.dma_start(
                    ach_t[b4][:].rearrange("p (r n) -> p r n", r=2),
                    ach[s].rearrange("r p n -> p r n"),
                ).then_inc(smp[s], 16)
                sync.dma_start(
                    fbR[b][:].rearrange("p (f n) -> p f n", f=3),
                    fbd[s].rearrange("f p n -> p f n"),
                ).then_inc(fsem[s], 16)

            def emit_out(s):
                b = s % NB
                sync.wait_ge(ssem, SSEM[(3, s)])
                for jj in range(2):
                    sync.dma_start(
                        outr[s][:, 512 * jj : 512 * jj + 512],
                        ob[b][:, 512 * jj : 512 * jj + 512],
                    ).then_inc(osem[s], 16)

            # startup: tiny cb first, then sample-0 x/ach so Pool can start,
            # then the two chunks of the constant table between loads
            sync.dma_start(cb[:], cbd[:]).then_inc(cbsem, 16)
            sync.dma_start(
                x_t[0][:].rearrange("p (c n) -> p c n", c=C),
                xt[0].rearrange("c (p n) -> p c n", n=128),
            ).then_inc(smp[0], 16)
            sync.dma_start(
                ach_t[0][:].rearrange("p (r n) -> p r n", r=2),
                ach[0].rearrange("r p n -> p r n"),
            ).then_inc(smp[0], 16)
            sync.dma_start(ca[:, 0:CA1_COLS], cad[:, 0:CA1_COLS]).then_inc(csem, 16)
            sync.dma_start(
                fbR[0][:].rearrange("p (f n) -> p f n", f=3),
                fbd[0].rearrange("f p n -> p f n"),
            ).then_inc(fsem[0], 16)
            emit_in(1)
            sync.dma_start(ca[:, CA1_COLS:], cad[:, CA1_COLS:]).then_inc(c2sem, 16)
            emit_in(2)
            emit_in(3)
            for s in range(SPC):
                if s + 4 < SPC:
                    emit_in(s + 4)
                emit_out(s)

        @block.gpsimd
        def _(gp):
            for kind, s in gp_order:
                b4 = s % 4
                gp.wait_ge(smp[s], 32)
                if s >= 4:
                    gp.wait_ge(psem, PSEM[(0, s - 4)])  # A_t[b4] free
                xv = x_t[b4][:].rearrange("p (c n) -> p c n", c=C)
                nc.gpsimd.tensor_tensor(
                    A_t[b4][:, 0:1024].rearrange("p (c n) -> p c n", c=C),
                    xv,
                    bcast8(ach_t[b4][:, 0:128]),
                    AluOp.mult,
                ).then_inc(gsem, 1)
                nc.gpsimd.tensor_tensor(
                    A_t[b4][:, 1024:2048].rearrange("p (c n) -> p c n", c=C),
                    xv,
                    bcast8(ach_t[b4][:, 128:256]),
                    AluOp.mult,
                ).then_inc(gsem, 1)

        @block.vector
        def _(vector):
            def chpages(ap):
                v_ = ap.rearrange("p (c u) -> p c u", c=C)
                return v_[:, :, 0:128], v_[:, :, 128:256]

            def prpages(ap):
                # s2 output layout: 4 q-blocks of (re 2ch*128 | im 2ch*128)
                v_ = ap.rearrange("p (q r u) -> p q r u", q=4, r=2)
                return v_[:, :, 0, :], v_[:, :, 1, :]  # [128, 4, 256] each

            def flat8(ap):
                return ap.rearrange("p (c u) -> p c u", c=C)

            def flat4(ap):
                return ap.rearrange("p (q u) -> p q u", q=4)

            def bcast4x2(tab):
                # [128,128] -> [128, 4, 2, 128] for the q-block × 2ch layout
                return tab.rearrange("p (o q u) -> p o q u", o=1, q=1).broadcast_to(
                    (128, 4, 2, 128)
                )

            for s0 in (0, 1):
                vector.wait_ge(smp[s0], 32)
                xv = x_t[s0][:].rearrange("p (c n) -> p c n", c=C)
                nc.vector.tensor_tensor(
                    A_t[s0][:, 0:1024].rearrange("p (c n) -> p c n", c=C),
                    xv,
                    bcast8(ach_t[s0][:, 0:128]),
                    AluOp.mult,
                ).then_inc(vsem, 1)
                nc.vector.tensor_tensor(
                    A_t[s0][:, 1024:2048].rearrange("p (c n) -> p c n", c=C),
                    xv,
                    bcast8(ach_t[s0][:, 128:256]),
                    AluOp.mult,
                ).then_inc(vsem, 1)
            first_dve = [True]
            for grp, s in dve_order:
                if first_dve[0]:
                    vector.wait_ge(csem, 16)
                    first_dve[0] = False
                    first_l3 = [True]
                b = s % NB
                if grp == 0:
                    # L1 (fwd twiddle, Karatsuba planes) from Yf
                    vector.wait_ge(ssem, SSEM[(0, s)])
                    if s >= NB:
                        vector.wait_ge(psem, PSEM[(1, s - NB)])  # Pb/Qnb/Rb free
                    yre, yim = chpages(Yf[b][:])
                    nc.vector.tensor_tensor(
                        flat8(Pb[b][:]), yre, bcast8(ca[:, _WR : _WR + 128]), AluOp.mult
                    ).then_inc(vsem, 1)
                    nc.vector.tensor_tensor(
                        flat8(Qnb[b][:]),
                        yim,
                        bcast8(ca[:, _WNI : _WNI + 128]),
                        AluOp.mult,
                    ).then_inc(vsem, 1)
                    nc.vector.tensor_tensor(
                        flat8(S1[b][:]), yre, yim, AluOp.add
                    ).then_inc(vsem, 1)
                    vector.wait_ge(vsem, VOP[(grp, s, 3)])  # S1 drained
                    nc.vector.tensor_tensor(
                        flat8(Rb[b][:]),
                        flat8(S1[b][:]),
                        bcast8(ca[:, _WS : _WS + 128]),
                        AluOp.mult,
                    ).then_inc(vsem, 1)
                elif grp == 1:
                    # C-layer (Fa o Fb, Karatsuba planes) from Ff
                    vector.wait_ge(ssem, SSEM[(1, s)])
                    vector.wait_ge(fsem[s], 16)
                    if s >= NB:
                        vector.wait_ge(psem, PSEM[(2, s - NB)])  # plane bufs free
                    fre, fim = prpages(Ff[b][:])
                    frev = fre.rearrange("p q (c u) -> p q c u", c=2)
                    fimv = fim.rearrange("p q (c u) -> p q c u", c=2)
                    nc.vector.tensor_tensor(
                        flat4(CRb[b][:]).rearrange("p q (c u) -> p q c u", c=2),
                        frev,
                        bcast4x2(fbR[b][:, 0:128]),
                        AluOp.mult,
                    ).then_inc(vsem, 1)  # P2 = Far*Fbr
                    nc.vector.tensor_tensor(
                        flat4(CIb[b][:]).rearrange("p q (c u) -> p q c u", c=2),
                        fimv,
                        bcast4x2(fbR[b][:, 128:256]),
                        AluOp.mult,
                    ).then_inc(vsem, 1)  # Q2n = Fai*(-Fbi)
                    nc.vector.tensor_tensor(
                        flat4(M1[b][:]), fre, fim, AluOp.add
                    ).then_inc(vsem, 1)  # s2 = Far+Fai
                    vector.wait_ge(vsem, VOP[(grp, s, 3)])  # s2 drained
                    nc.vector.tensor_tensor(
                        flat4(M2[b][:]).rearrange("p q (c u) -> p q c u", c=2),
                        flat4(M1[b][:]).rearrange("p q (c u) -> p q c u", c=2),
                        bcast4x2(fbR[b][:, 256:384]),
                        AluOp.mult,
                    ).then_inc(vsem, 1)  # R2 = s2*(Fbr+Fbi)
                else:
                    # L3 (inv twiddle, Karatsuba planes) from Sf
                    if first_l3[0]:
                        vector.wait_ge(c2sem, 16)
                        first_l3[0] = False
                    vector.wait_ge(ssem, SSEM[(2, s)])
                    if s >= NB:
                        vector.wait_ge(psem, PSEM[(3, s - NB)])  # P3b/.. free
                    sre, sim_ = chpages(Sf[b][:])
                    nc.vector.tensor_tensor(
                        flat8(P3b[b][:]),
                        sre,
                        bcast8(ca[:, _W2R : _W2R + 128]),
                        AluOp.mult,
                    ).then_inc(vsem, 1)
                    nc.vector.tensor_tensor(
                        flat8(Q3nb[b][:]),
                        sim_,
                        bcast8(ca[:, _W2NI : _W2NI + 128]),
                        AluOp.mult,
                    ).then_inc(vsem, 1)
                    nc.vector.tensor_tensor(
                        flat8(S3[b][:]), sre, sim_, AluOp.add
                    ).then_inc(vsem, 1)
                    vector.wait_ge(vsem, VOP[(grp, s, 3)])  # S3 drained
                    nc.vector.tensor_tensor(
                        flat8(R3b[b][:]),
                        flat8(S3[b][:]),
                        bcast8(ca[:, _W2S : _W2S + 128]),
                        AluOp.mult,
                    ).then_inc(vsem, 1)

        @block.tensor
        def _(tensor):
            mm = nc.tensor.matmul
            first_pe = [True]
            first_is1 = [True]

            def phase_s1(s):
                ps = psR[s % 2]
                if first_pe[0]:
                    tensor.wait_ge(cbsem, 16)  # cb loaded
                    first_pe[0] = False
                if s < 2:
                    tensor.wait_ge(vsem, 2 * (s + 1))  # startup a-planes on DVE
                else:
                    tensor.wait_ge(gsem, GSEM[(0, s)])
                if s >= NB:
                    tensor.wait_ge(ssem, SSEM[(3, s - NB)])  # region free
                b4 = s % 4
                for c in range(C):
                    o = ps[:, 256 * c : 256 * c + 256]
                    mm(
                        o,
                        A_t[b4][:, 128 * c : 128 * c + 128],
                        cb[:, 0:256],
                        start=True,
                        stop=False,
                    )
                    i = mm(
                        o,
                        A_t[b4][:, 1024 + 128 * c : 1024 + 128 * c + 128],
                        cb[:, 256:512],
                        start=False,
                        stop=True,
                    )
                    if c == C - 1:
                        i.then_inc(psem, 1)

            def phase_s2(s):
                b = s % NB
                ps = psR[s % 2]
                tensor.wait_ge(vsem, VOP[(0, s, 1)])  # Pb ready
                tensor.wait_ge(csem, 16)  # ca chunk1 loaded
                tensor.wait_ge(ssem, SSEM[(0, s)])  # region free after evacY
                srcs = [
                    (Pb[b], _F[0], 0, True, False, None),
                    (Qnb[b], _F[0], 256, False, False, VOP[(0, s, 2)]),
                    (Qnb[b], _F[1], 0, False, False, None),
                    (Pb[b], _F[2], 256, False, False, None),
                    (Rb[b], _F[3], 0, False, False, VOP[(0, s, 4)]),
                    (Rb[b], _F[4], 256, False, True, None),
                ]
                for wi, (buf, fofs, oofs, st, sp, wv) in enumerate(srcs):
                    if wv is not None:
                        tensor.wait_ge(vsem, wv)
                    for q in range(4):
                        i = mm(
                            ps[:, 512 * q + oofs : 512 * q + oofs + 256],
                            ca[:, fofs : fofs + 128],
                            buf[:, 256 * q : 256 * q + 256],
                            start=st,
                            stop=sp,
                        )
                        if wi == 5 and q == 3:
                            i.then_inc(psem, 1)

            def phase_is1(s):
                b = s % NB
                ps = psR[s % 2]
                if first_is1[0]:
                    tensor.wait_ge(c2sem, 16)  # H tables in the 2nd const DMA
                    first_is1[0] = False
                tensor.wait_ge(vsem, VOP[(1, s, 1)])  # P2 ready
                tensor.wait_ge(ssem, SSEM[(1, s)])
                for c in range(C):
                    # even channel opens its bank; odd writes the other half
                    mm(
                        ps[:, 256 * c : 256 * c + 256],
                        CRb[b][:, 128 * c : 128 * c + 128],
                        ca[:, _HA : _HA + 256],
                        start=(c % 2 == 0),
                        stop=False,
                    )
                tensor.wait_ge(vsem, VOP[(1, s, 2)])  # Q2n ready
                for c in range(C):
                    mm(
                        ps[:, 256 * c : 256 * c + 256],
                        CIb[b][:, 128 * c : 128 * c + 128],
                        ca[:, _HB : _HB + 256],
                        start=False,
                        stop=False,
                    )
                tensor.wait_ge(vsem, VOP[(1, s, 4)])  # R2 ready
                for c in range(C):
                    i = mm(
                        ps[:, 256 * c : 256 * c + 256],
                        M2[b][:, 128 * c : 128 * c + 128],
                        ca[:, _HR : _HR + 256],
                        start=False,
                        stop=(c % 2 == 1),
                    )
                    if c == C - 1:
                        i.then_inc(psem, 1)

            def phase_is2(s):
                b = s % NB
                ps = psR[s % 2]
                tensor.wait_ge(vsem, VOP[(2, s, 1)])  # P3b ready
                tensor.wait_ge(ssem, SSEM[(2, s)])
                for c in range(C):
                    mm(
                        ps[:, 128 * c : 128 * c + 128],
                        P3b[b][:, 128 * c : 128 * c + 128],
                        ca[:, _KP : _KP + 128],
                        start=(c % 4 == 0),
                        stop=False,
                    )
                tensor.wait_ge(vsem, VOP[(2, s, 2)])  # Q3nb ready
                for c in range(C):
                    mm(
                        ps[:, 128 * c : 128 * c + 128],
                        Q3nb[b][:, 128 * c : 128 * c + 128],
                        ca[:, _KQ : _KQ + 128],
                        start=False,
                        stop=False,
                    )
                tensor.wait_ge(vsem, VOP[(2, s, 4)])  # R3b ready
                for c in range(C):
                    i = mm(
                        ps[:, 128 * c : 128 * c + 128],
                        R3b[b][:, 128 * c : 128 * c + 128],
                        ca[:, _KR : _KR + 128],
                        start=False,
                        stop=(c % 4 == 3),
                    )
                    if c == C - 1:
                        i.then_inc(psem, 1)

            phase_fns = [phase_s1, phase_s2, phase_is1, phase_is2]
            for ph, s in pe_order:
                phase_fns[ph](s)

        @block.scalar
        def _(scalar):
            for ph, s in act_order:
                b = s % NB
                ps = psR[s % 2]
                scalar.wait_ge(psem, PSEM[(ph, s)])
                if ph == 0:
                    nc.scalar.copy(Yf[b][:], ps[:, 0:2048]).then_inc(ssem, 1)
                elif ph == 1:
                    nc.scalar.copy(Ff[b][:], ps[:, 0:2048]).then_inc(ssem, 1)
                elif ph == 2:
                    nc.scalar.copy(Sf[b][:], ps[:, 0:2048]).then_inc(ssem, 1)
                else:
                    if s >= NB:
                        scalar.wait_ge(osem[s - NB], 32)
                    nc.scalar.copy(ob[b][:], ps[:, 0:1024]).then_inc(ssem, 1)

    for t in reversed(ctx_list):
        t.__exit__(None, None, None)

    return nc


def _get_module():
    if "nc" not in _MODULE_CACHE:
        _MODULE_CACHE["nc"] = _build_module()
    return _MODULE_CACHE["nc"]


# ---------------------------------------------------------------------------
# host side
# ---------------------------------------------------------------------------


def _host_tables(rpm):
    """Per-sample chirp tables + Fb planes (un-replicated)."""
    pad = np.floor((RES * 60.0 / rpm.astype(np.float64) - TS) * SF).astype(np.int64)
    n_arr = L + pad
    t = np.arange(L, dtype=np.int64)
    m = np.arange(M, dtype=np.int64)
    mm = np.minimum(m, M - m)

    ach = np.empty((B, 2, 64, 128), np.float16)
    fbp = np.empty((B, 3, 128, 128), np.float16)
    for b in range(B):
        n = int(n_arr[b])
        two_n = 2 * n
        ph = np.pi * ((t * t) % two_n) / n
        ach[b, 0] = np.cos(ph).astype(np.float16).reshape(64, 128)
        ach[b, 1] = (-np.sin(ph)).astype(np.float16).reshape(64, 128)
        phb = np.pi * ((mm * mm) % two_n) / n
        Fb = np.fft.fft(np.exp(1j * phb)).reshape(128, 128) * FBSCALE
        fbp[b, 0] = Fb.real.astype(np.float16)
        fbp[b, 1] = (-Fb.imag).astype(np.float16)
        fbp[b, 2] = (Fb.real + Fb.imag).astype(np.float16)
    return ach, fbp


LAST_EXEC_WALL_NS = [None]


def kernel(inputs, rpm):
    inputs = np.ascontiguousarray(inputs, dtype=np.float32)  # [B, L, C]
    rpm = np.ascontiguousarray(rpm, dtype=np.float32)

    ca, cb = _consts()
    ach, fbp = _host_tables(rpm)
    xt = np.ascontiguousarray(inputs.transpose(0, 2, 1)).astype(np.float16)

    nc = _get_module()
    in_maps = []
    for g in range(NCORES):
        s0 = g * SPC
        in_maps.append(
            {
                "xt": xt[s0 : s0 + SPC],
                "ach": ach[s0 : s0 + SPC],
                "fbd": fbp[s0 : s0 + SPC],
                "cad": ca,
                "cbd": cb,
            }
        )

    import time

    from concourse.bass_utils import run_bass_kernel_spmd

    t0 = time.perf_counter_ns()
    res = run_bass_kernel_spmd(nc, in_maps, list(range(NCORES)))
    LAST_EXEC_WALL_NS[0] = time.perf_counter_ns() - t0

    out = np.empty((B, L, C), np.float32)
    for g in range(NCORES):
        planes = np.asarray(res.results[g]["outr"], np.float32)  # [SPC, 128, 1024]
        arr = planes.reshape(SPC, 128, C, 2, 64)  # [s, m1, c, re|im, m2]
        mag = np.hypot(arr[:, :, :, 0, :], arr[:, :, :, 1, :])  # [s, m1, c, m2]
        # conv index k = m1 + 128*m2  ->  out[s, k, c]
        out[g * SPC : (g + 1) * SPC] = (
            mag.transpose(0, 3, 1, 2).reshape(SPC, L, C)
        )
    return out


# revision 17
# speedup vs baseline: 1.2942x; 1.1714x over previous
"""EngineOrderFFT (Bluestein chirp-Z, fixed M=16384) Trainium2 kernel, V2.

Strategy
--------
Pure data parallelism: batch dim B=64 split across 8 NeuronCores
(8 samples/core). Each sample's variable-length DFT (length n_b) is a
Bluestein transform with fixed FFT size M=16384 = 128*128; each
16384-point (i)FFT is a two-stage Cooley-Tukey factorization executed as
fp16 matmuls on the tensor engine.

V2 changes vs V1:
  * is2 (inverse stage 2) runs in data-as-lhsT form: per channel the
    twiddled planes P3/Q3n/R3 [k2,m1] are the stationary operand and the
    Karatsuba combine tables K_P/K_Q/K_R [k2, m2(re|im)] are the moving
    operand -> 3072 output columns/sample instead of 6144, and the output
    lands as [m1, ch*(re64|im64)] = [128, 1024] (half the PSUM, cheaper
    evacuation).
  * All chirp/twiddle/Fb tables are stored un-replicated; the elementwise
    products broadcast them across the 8 channel pages with 0-stride APs
    (same DVE cost, 8x less DMA + SBUF).

Engine split per sample (8 channels batched in every instruction):
  sync   in/out DMAs
  gpsimd a-planes (x*chirp)
  PE     4 matmul stages (s1 16mm/4096col, s2 24mm/6144, is1 24mm/6144,
         is2 24mm/3072)
  ACT    PSUM -> fp16 SBUF evacuation after each stage
  DVE    twiddle/pointwise Karatsuba product planes (fp16 2x mode)

|conv[k]| is computed on the host from the shipped re/im planes.
"""
import numpy as np

SF, RES, TS = 8192, 40, 1
B, L, C = 64, 8192, 8
M = 16384
NCORES = 8
SPC = B // NCORES  # samples per core

FBSCALE = 1.0 / 32.0
HSCALE = 1.0 / 16.0
KSCALE = 1.0 / 32.0  # HSCALE*KSCALE = (1/M) * (1/FBSCALE)

# ---------------------------------------------------------------------------
# constant tables (input-independent)
# ---------------------------------------------------------------------------


def _f16(x):
    return np.ascontiguousarray(x, dtype=np.float16)


def _build_const_tables():
    j = np.arange(128)
    D = np.exp(-2j * np.pi * np.outer(j, j) / 128.0)  # symmetric
    Dc = np.conj(D)
    Wt = np.exp(-2j * np.pi * np.outer(j, j) / M)  # fwd twiddle [n1,k2]
    W2 = np.conj(Wt)  # inv twiddle
    Dr, Di = D.real, D.imag
    Hr, Hi = (Dc * HSCALE).real, (Dc * HSCALE).imag
    Kr, Ki = (Dc * KSCALE).real[:, :64], (Dc * KSCALE).imag[:, :64]

    cols = []
    # chunk 1 (cols 0:1024): W twiddle (L1), F tables (s2)
    cols += [Wt.real, -Wt.imag, Wt.real + Wt.imag]  # _WR,_WNI,_WS [128,128]
    cols += [Dr + Di, Dr - Di, Di - Dr, -Di, Dr]  # F1,F2,F2n,F3,F4 [128,128]
    # chunk 2 (cols 1024:2560): H (is1), W2 (L3), K (is2)
    cols += [np.concatenate([Hr + Hi, Hi - Hr], 1)]  # H_P [128,256]
    cols += [np.concatenate([Hr - Hi, Hi + Hr], 1)]  # H_Q
    cols += [np.concatenate([-Hi, Hr], 1)]  # H_R
    cols += [W2.real, -W2.imag, W2.real + W2.imag]  # _W2R,_W2NI,_W2S
    cols += [np.concatenate([Kr + Ki, Ki - Kr], 1)]  # K_P [128,128]
    cols += [np.concatenate([Kr - Ki, Ki + Kr], 1)]  # K_Q
    cols += [np.concatenate([-Ki, Kr], 1)]  # K_R
    ca = _f16(np.concatenate(cols, axis=1))

    cb = _f16(
        np.concatenate([Dr[:64], Di[:64], -Di[:64], Dr[:64]], axis=1)
    )  # [64, 512] = Dtab1|Dtab2
    return ca, cb


# column offsets in ca
_WR, _WNI, _WS = 0, 128, 256
_F = [384, 512, 640, 768, 896]  # F1,F2,F2n,F3,F4
_HA, _HB, _HR = 1024, 1280, 1536
_W2R, _W2NI, _W2S = 1792, 1920, 2048
_KP, _KQ, _KR = 2176, 2304, 2432
CA1_COLS = 1024
CA_COLS = 2560

_CONST_CACHE = {}


def _consts():
    if "ca" not in _CONST_CACHE:
        ca, cb = _build_const_tables()
        assert ca.shape[1] == CA_COLS, ca.shape
        _CONST_CACHE["ca"] = ca
        _CONST_CACHE["cb"] = cb
    return _CONST_CACHE["ca"], _CONST_CACHE["cb"]


# ---------------------------------------------------------------------------
# device module
# ---------------------------------------------------------------------------

_MODULE_CACHE = {}


def _build_module():
    import concourse.bass as bass
    from concourse import mybir

    dt = mybir.dt
    NB = 2  # per-sample buffer depth

    nc = bass.Bass("TRN2", target_bir_lowering=False, debug=False)

    xt = nc.dram_tensor("xt", [SPC, C, L], dt.float16, kind="ExternalInput").ap()
    # chirp tables (un-replicated): [SPC, 2, 64, 128] = (cos, -sin)
    ach = nc.dram_tensor("ach", [SPC, 2, 64, 128], dt.float16, kind="ExternalInput").ap()
    # Fb planes (un-replicated): [SPC, 3, 128, 128] = (Fbr, -Fbi, Fbr+Fbi)*FBSCALE
    fbd = nc.dram_tensor("fbd", [SPC, 3, 128, 128], dt.float16, kind="ExternalInput").ap()
    cad = nc.dram_tensor("cad", [128, CA_COLS], dt.float16, kind="ExternalInput").ap()
    cbd = nc.dram_tensor("cbd", [64, 512], dt.float16, kind="ExternalInput").ap()
    # out: [SPC, 128(m1), 8ch * (re64|im64)(m2)] ; conv[m1+128*m2]
    outr = nc.dram_tensor("outr", [SPC, 128, 1024], dt.float16, kind="ExternalOutput").ap()

    ctx_list = []

    def sb(name, shape, dtype=None):
        t = nc.sbuf_tensor(name, shape, dtype or mybir.dt.float16)
        ap = t.__enter__()
        ctx_list.append(t)
        return ap

    def psum(name, shape):
        t = nc.psum_tensor(name, shape, mybir.dt.float32)
        ap = t.__enter__()
        ctx_list.append(t)
        return ap

    ca = sb("ca", [128, CA_COLS])
    cb = sb("cb", [64, 512])
    ND = 6  # input/a-plane buffer depth (Pool must run well ahead)
    x_t = [sb(f"x{i}", [64, 1024]) for i in range(ND)]
    ach_t = [sb(f"ach{i}", [64, 256]) for i in range(ND)]
    fbR = [sb(f"fbR{i}", [128, 384]) for i in range(ND)]
    A_t = [sb(f"A{i}", [64, 2048]) for i in range(ND)]
    Yf = [sb(f"Yf{i}", [128, 2048]) for i in range(NB)]
    Pb = [sb(f"Pb{i}", [128, 1024]) for i in range(NB)]
    Qnb = [sb(f"Qnb{i}", [128, 1024]) for i in range(NB)]
    Rb = [sb(f"Rb{i}", [128, 1024]) for i in range(NB)]
    Ff = [sb(f"Ff{i}", [128, 2048]) for i in range(NB)]
    CRb = [sb(f"CRb{i}", [128, 1024]) for i in range(NB)]
    CIb = [sb(f"CIb{i}", [128, 1024]) for i in range(NB)]
    Sf = [sb(f"Sf{i}", [128, 2048]) for i in range(NB)]
    P3b = [sb(f"P3b{i}", [128, 1024]) for i in range(NB)]
    Q3nb = [sb(f"Q3nb{i}", [128, 1024]) for i in range(NB)]
    R3b = [sb(f"R3b{i}", [128, 1024]) for i in range(NB)]
    ob = [sb(f"ob{i}", [128, 1024]) for i in range(NB)]
    S1 = [sb(f"S1_{i}", [128, 1024]) for i in range(NB)]
    S3 = [sb(f"S3_{i}", [128, 1024]) for i in range(NB)]
    M1 = [sb(f"M1_{i}", [128, 1024]) for i in range(NB)]
    M2 = [sb(f"M2_{i}", [128, 1024]) for i in range(NB)]

    # two 4-bank psum regions; samples alternate regions by parity
    psR = [psum("psR0", [128, 2048]), psum("psR1", [128, 2048])]

    csem = nc.alloc_semaphore("csem")
    c2sem = nc.alloc_semaphore("c2sem")
    cbsem = nc.alloc_semaphore("cbsem")
    smp = [nc.alloc_semaphore(f"smp{i}") for i in range(SPC)]
    osem = [nc.alloc_semaphore(f"osem{i}") for i in range(SPC)]
    fsem = [nc.alloc_semaphore(f"fsem{i}") for i in range(SPC)]
    vsem = nc.alloc_semaphore("vsem")
    psem = nc.alloc_semaphore("psem")
    ssem = nc.alloc_semaphore("ssem")
    gsem = nc.alloc_semaphore("gsem")

    # ---- emission orders (pair-interleaved, half-split) and sem tables ----
    # Every evac / product layer / PE phase is split into channel halves
    # h=0 (ch 0-3) and h=1 (ch 4-7) so the evac->product->matmul chain per
    # half is ~1.7us instead of ~3us and PE stays fed.
    pairs = [(2 * p, 2 * p + 1) for p in range(SPC // 2)]

    pe_order = []   # (phase, s, h): half-sample units
    act_order = []  # (evac, s, h)
    dve_order = []  # (group, s), group in 0..2 (L1, CL, L3); halves inside
    gp_order = []   # (kind, s), kind 0=a-planes
    # Software pipeline: pair p's is2 window (short, product-gated units) is
    # filled with pair p+1's s1 units, and ACT mirrors that order so the ob
    # evacs (which free s1 regions) and the next Yf evacs (which feed the L1
    # chain) land just-in-time.
    for p, (sa, sb_) in enumerate(pairs):
        if p == 0:
            pe_order += [(0, sa, 0), (0, sa, 1), (0, sb_, 0), (0, sb_, 1)]
            act_order += [(0, sa, 0), (0, sa, 1), (0, sb_, 0), (0, sb_, 1)]
        for ph in (1, 2):
            pe_order += [(ph, sa, 0), (ph, sa, 1), (ph, sb_, 0), (ph, sb_, 1)]
            act_order += [(ph, sa, 0), (ph, sa, 1), (ph, sb_, 0), (ph, sb_, 1)]
        if p + 1 < len(pairs):
            na, nb_ = pairs[p + 1]
            pe_order += [
                (3, sa, 0), (3, sa, 1), (0, na, 0), (3, sb_, 0),
                (0, na, 1), (3, sb_, 1), (0, nb_, 0), (0, nb_, 1),
            ]
            act_order += [
                (3, sa, 0), (3, sa, 1), (0, na, 0), (3, sb_, 0),
                (0, na, 1), (3, sb_, 1), (0, nb_, 0), (0, nb_, 1),
            ]
        else:
            pe_order += [(3, sa, 0), (3, sa, 1), (3, sb_, 0), (3, sb_, 1)]
            act_order += [(3, sa, 0), (3, sa, 1), (3, sb_, 0), (3, sb_, 1)]
        for g in range(3):
            dve_order += [(g, sa), (g, sb_)]
        gp_order += [(0, sa), (0, sb_)]
    gp_order = [e for e in gp_order if e[1] >= 2]

    # PE increments psem once per half-phase, in emission order.
    PSEM = {}
    for i, key in enumerate(pe_order):
        PSEM[key] = i + 1
    SSEM = {}
    for i, key in enumerate(act_order):
        SSEM[key] = i + 1
    GSEM = {}
    g = 0
    for kind, s in gp_order:
        g += 2
        GSEM[(kind, s)] = g
    # DVE op positions per (group, half): P(+1), Qn(+2), S(+3), R(+4)
    VSEM = {}
    VOP = {}
    v = 4  # 4 startup a-plane ops on DVE (samples 0,1)
    for grp, s in dve_order:
        for h in range(2):
            for k in range(1, 5):
                VOP[(grp, s, h, k)] = v + 4 * h + k
        v += 8
        VSEM[(grp, s)] = v

    AluOp = mybir.AluOpType

    def bcast8(tab):
        """[P,128] table -> [P, 8, 128] zero-stride channel broadcast."""
        p = tab.shape[0]
        return tab.rearrange("p (o u) -> p o u", o=1).broadcast_to((p, 8, 128))

    with nc.Block() as block:

        @block.sync
        def _(sync):
            def emit_in(s):
                b = s % NB
                b4 = s % ND
                if s >= ND:
                    if s - ND < 2:
                        sync.wait_ge(vsem, 2 * (s - ND + 1))
                    else:
                        sync.wait_ge(gsem, GSEM[(0, s - ND)])
                if s >= ND:
                    sync.wait_ge(vsem, VSEM[(1, s - ND)])
                sync.dma_start(
                    x_t[b4][:].rearrange("p (c n) -> p c n", c=C),
                    xt[s].rearrange("c (p n) -> p c n", n=128),
                ).then_inc(smp[s], 16)
                sync.dma_start(
                    ach_t[b4][:].rearrange("p (r n) -> p r n", r=2),
                    ach[s].rearrange("r p n -> p r n"),
                ).then_inc(smp[s], 16)
                sync.dma_start(
                    fbR[s % ND][:].rearrange("p (f n) -> p f n", f=3),
                    fbd[s].rearrange("f p n -> p f n"),
                ).then_inc(fsem[s], 16)

            def emit_out(s):
                b = s % NB
                for jj in range(2):
                    sync.wait_ge(ssem, SSEM[(3, s, jj)])
                    sync.dma_start(
                        outr[s][:, 512 * jj : 512 * jj + 512],
                        ob[b][:, 512 * jj : 512 * jj + 512],
                    ).then_inc(osem[s], 16)

            # startup: tiny cb first, then sample-0 x/ach so Pool can start,
            # then the two chunks of the constant table between loads
            sync.dma_start(cb[:], cbd[:]).then_inc(cbsem, 16)
            sync.dma_start(
                x_t[0][:].rearrange("p (c n) -> p c n", c=C),
                xt[0].rearrange("c (p n) -> p c n", n=128),
            ).then_inc(smp[0], 16)
            sync.dma_start(
                ach_t[0][:].rearrange("p (r n) -> p r n", r=2),
                ach[0].rearrange("r p n -> p r n"),
            ).then_inc(smp[0], 16)
            sync.dma_start(ca[:, 0:CA1_COLS], cad[:, 0:CA1_COLS]).then_inc(csem, 16)
            sync.dma_start(
                fbR[0][:].rearrange("p (f n) -> p f n", f=3),
                fbd[0].rearrange("f p n -> p f n"),
            ).then_inc(fsem[0], 16)
            emit_in(1)
            sync.dma_start(ca[:, CA1_COLS:], cad[:, CA1_COLS:]).then_inc(c2sem, 16)
            for s_ in range(2, ND):
                emit_in(s_)
            for s in range(SPC):
                if s + ND < SPC:
                    emit_in(s + ND)
                emit_out(s)

        @block.gpsimd
        def _(gp):
            for kind, s in gp_order:
                b4 = s % ND
                gp.wait_ge(smp[s], 32)
                if s >= ND:
                    gp.wait_ge(psem, PSEM[(0, s - ND, 1)])  # A_t[b4] free
                xv = x_t[b4][:].rearrange("p (c n) -> p c n", c=C)
                nc.gpsimd.tensor_tensor(
                    A_t[b4][:, 0:1024].rearrange("p (c n) -> p c n", c=C),
                    xv,
                    bcast8(ach_t[b4][:, 0:128]),
                    AluOp.mult,
                ).then_inc(gsem, 1)
                nc.gpsimd.tensor_tensor(
                    A_t[b4][:, 1024:2048].rearrange("p (c n) -> p c n", c=C),
                    xv,
                    bcast8(ach_t[b4][:, 128:256]),
                    AluOp.mult,
                ).then_inc(gsem, 1)

        @block.vector
        def _(vector):
            def chpages(ap):
                v_ = ap.rearrange("p (c u) -> p c u", c=C)
                return v_[:, :, 0:128], v_[:, :, 128:256]

            def prpages(ap):
                # s2 output layout: 4 q-blocks of (re 2ch*128 | im 2ch*128)
                v_ = ap.rearrange("p (q r u) -> p q r u", q=4, r=2)
                return v_[:, :, 0, :], v_[:, :, 1, :]  # [128, 4, 256] each

            def flat8(ap):
                return ap.rearrange("p (c u) -> p c u", c=C)

            def flat4(ap):
                return ap.rearrange("p (q u) -> p q u", q=4)

            def bcast4x2(tab):
                # [128,128] -> [128, 4, 2, 128] for the q-block × 2ch layout
                return tab.rearrange("p (o q u) -> p o q u", o=1, q=1).broadcast_to(
                    (128, 4, 2, 128)
                )

            for s0 in (0, 1):
                vector.wait_ge(smp[s0], 32)
                xv = x_t[s0][:].rearrange("p (c n) -> p c n", c=C)
                nc.vector.tensor_tensor(
                    A_t[s0][:, 0:1024].rearrange("p (c n) -> p c n", c=C),
                    xv,
                    bcast8(ach_t[s0][:, 0:128]),
                    AluOp.mult,
                ).then_inc(vsem, 1)
                nc.vector.tensor_tensor(
                    A_t[s0][:, 1024:2048].rearrange("p (c n) -> p c n", c=C),
                    xv,
                    bcast8(ach_t[s0][:, 128:256]),
                    AluOp.mult,
                ).then_inc(vsem, 1)
            def bc_h(tab, c_):
                # [128,128] table -> [128, c_, 128] channel-half broadcast
                return tab.rearrange("p (o u) -> p o u", o=1).broadcast_to(
                    (128, c_, 128)
                )

            first_dve = [True]
            for grp, s in dve_order:
                if first_dve[0]:
                    vector.wait_ge(csem, 16)
                    first_dve[0] = False
                    first_l3 = [True]
                b = s % NB
                if grp == 0:
                    # L1 (fwd twiddle, Karatsuba planes) from Yf, by halves
                    if s >= NB:
                        vector.wait_ge(psem, PSEM[(1, s - NB, 1)])  # bufs free
                    yre, yim = chpages(Yf[b][:])
                    for h in range(2):
                        vector.wait_ge(ssem, SSEM[(0, s, h)])
                        c4 = slice(4 * h, 4 * h + 4)
                        yre_h, yim_h = yre[:, c4, :], yim[:, c4, :]
                        o = slice(512 * h, 512 * h + 512)
                        nc.vector.tensor_tensor(
                            flat8(Pb[b][:])[:, c4, :],
                            yre_h,
                            bc_h(ca[:, _WR : _WR + 128], 4),
                            AluOp.mult,
                        ).then_inc(vsem, 1)
                        nc.vector.tensor_tensor(
                            flat8(Qnb[b][:])[:, c4, :],
                            yim_h,
                            bc_h(ca[:, _WNI : _WNI + 128], 4),
                            AluOp.mult,
                        ).then_inc(vsem, 1)
                        nc.vector.tensor_tensor(
                            flat8(S1[b][:])[:, c4, :], yre_h, yim_h, AluOp.add
                        ).then_inc(vsem, 1)
                        vector.wait_ge(vsem, VOP[(grp, s, h, 3)])  # S1h drained
                        nc.vector.tensor_tensor(
                            flat8(Rb[b][:])[:, c4, :],
                            flat8(S1[b][:])[:, c4, :],
                            bc_h(ca[:, _WS : _WS + 128], 4),
                            AluOp.mult,
                        ).then_inc(vsem, 1)
                elif grp == 1:
                    # C-layer (Fa o Fb, Karatsuba planes) from Ff, by halves
                    vector.wait_ge(fsem[s], 16)
                    if s >= NB:
                        vector.wait_ge(psem, PSEM[(2, s - NB, 1)])  # bufs free
                    fre, fim = prpages(Ff[b][:])
                    for h in range(2):
                        vector.wait_ge(ssem, SSEM[(1, s, h)])
                        q2 = slice(2 * h, 2 * h + 2)
                        fre_h = fre[:, q2, :].rearrange("p q (c u) -> p q c u", c=2)
                        fim_h = fim[:, q2, :].rearrange("p q (c u) -> p q c u", c=2)
                        crv = flat4(CRb[b][:])[:, q2, :].rearrange(
                            "p q (c u) -> p q c u", c=2
                        )
                        civ = flat4(CIb[b][:])[:, q2, :].rearrange(
                            "p q (c u) -> p q c u", c=2
                        )
                        m1v = flat4(M1[b][:])[:, q2, :]
                        m2v = flat4(M2[b][:])[:, q2, :].rearrange(
                            "p q (c u) -> p q c u", c=2
                        )

                        def bc22(tab):
                            return tab.rearrange(
                                "p (o q u) -> p o q u", o=1, q=1
                            ).broadcast_to((128, 2, 2, 128))

                        nc.vector.tensor_tensor(
                            crv, fre_h, bc22(fbR[s % ND][:, 0:128]), AluOp.mult
                        ).then_inc(vsem, 1)  # P2 = Far*Fbr
                        nc.vector.tensor_tensor(
                            civ, fim_h, bc22(fbR[s % ND][:, 128:256]), AluOp.mult
                        ).then_inc(vsem, 1)  # Q2n = Fai*(-Fbi)
                        nc.vector.tensor_tensor(
                            m1v, fre[:, q2, :], fim[:, q2, :], AluOp.add
                        ).then_inc(vsem, 1)  # s2 = Far+Fai
                        vector.wait_ge(vsem, VOP[(grp, s, h, 3)])  # s2h drained
                        nc.vector.tensor_tensor(
                            m2v,
                            flat4(M1[b][:])[:, q2, :].rearrange(
                                "p q (c u) -> p q c u", c=2
                            ),
                            bc22(fbR[s % ND][:, 256:384]),
                            AluOp.mult,
                        ).then_inc(vsem, 1)  # R2 = s2*(Fbr+Fbi)
                else:
                    # L3 (inv twiddle, Karatsuba planes) from Sf, by halves
                    if first_l3[0]:
                        vector.wait_ge(c2sem, 16)
                        first_l3[0] = False
                    if s >= NB:
                        vector.wait_ge(psem, PSEM[(3, s - NB, 1)])  # bufs free
                    sre, sim_ = chpages(Sf[b][:])
                    for h in range(2):
                        vector.wait_ge(ssem, SSEM[(2, s, h)])
                        c4 = slice(4 * h, 4 * h + 4)
                        sre_h, sim_h = sre[:, c4, :], sim_[:, c4, :]
                        nc.vector.tensor_tensor(
                            flat8(P3b[b][:])[:, c4, :],
                            sre_h,
                            bc_h(ca[:, _W2R : _W2R + 128], 4),
                            AluOp.mult,
                        ).then_inc(vsem, 1)
                        nc.vector.tensor_tensor(
                            flat8(Q3nb[b][:])[:, c4, :],
                            sim_h,
                            bc_h(ca[:, _W2NI : _W2NI + 128], 4),
                            AluOp.mult,
                        ).then_inc(vsem, 1)
                        nc.vector.tensor_tensor(
                            flat8(S3[b][:])[:, c4, :], sre_h, sim_h, AluOp.add
                        ).then_inc(vsem, 1)
                        vector.wait_ge(vsem, VOP[(grp, s, h, 3)])  # S3h drained
                        nc.vector.tensor_tensor(
                            flat8(R3b[b][:])[:, c4, :],
                            flat8(S3[b][:])[:, c4, :],
                            bc_h(ca[:, _W2S : _W2S + 128], 4),
                            AluOp.mult,
                        ).then_inc(vsem, 1)

        @block.tensor
        def _(tensor):
            mm = nc.tensor.matmul
            first_pe = [True]
            first_is1 = [True]

            def phase_s1(s, h):
                rg = psR[s % 2][:, 1024 * h : 1024 * h + 1024]
                if first_pe[0]:
                    tensor.wait_ge(cbsem, 16)  # cb loaded
                    first_pe[0] = False
                if h == 0:
                    if s < 2:
                        tensor.wait_ge(vsem, 2 * (s + 1))  # startup a-planes
                    else:
                        tensor.wait_ge(gsem, GSEM[(0, s)])
                if s >= NB:
                    # region free once ob evac half h of s-NB done
                    tensor.wait_ge(ssem, SSEM[(3, s - NB, h)])
                b4 = s % ND
                for c in range(4 * h, 4 * h + 4):
                    o = rg[:, 256 * (c - 4 * h) : 256 * (c - 4 * h) + 256]
                    mm(
                        o,
                        A_t[b4][:, 128 * c : 128 * c + 128],
                        cb[:, 0:256],
                        start=True,
                        stop=False,
                    )
                    i = mm(
                        o,
                        A_t[b4][:, 1024 + 128 * c : 1024 + 128 * c + 128],
                        cb[:, 256:512],
                        start=False,
                        stop=True,
                    )
                    if c % 4 == 3:
                        i.then_inc(psem, 1)

            def phase_s2(s, h):
                b = s % NB
                rg = psR[s % 2][:, 1024 * h : 1024 * h + 1024]
                if h == 0:
                    tensor.wait_ge(csem, 16)  # ca chunk1 loaded
                srcs = [
                    (Pb[b], _F[0], 0, True, False, 1),
                    (Qnb[b], _F[0], 256, False, False, 2),
                    (Qnb[b], _F[1], 0, False, False, None),
                    (Pb[b], _F[2], 256, False, False, None),
                    (Rb[b], _F[3], 0, False, False, 4),
                    (Rb[b], _F[4], 256, False, True, None),
                ]
                for wi, (buf, fofs, oofs, st, sp, wk) in enumerate(srcs):
                    if wk is not None:
                        tensor.wait_ge(vsem, VOP[(0, s, h, wk)])
                    for ql in range(2):
                        q = 2 * h + ql
                        i = mm(
                            rg[:, 512 * ql + oofs : 512 * ql + oofs + 256],
                            ca[:, fofs : fofs + 128],
                            buf[:, 256 * q : 256 * q + 256],
                            start=st,
                            stop=sp,
                        )
                        if wi == 5 and ql == 1:
                            i.then_inc(psem, 1)

            def phase_is1(s, h):
                b = s % NB
                rg = psR[s % 2][:, 1024 * h : 1024 * h + 1024]
                if first_is1[0]:
                    tensor.wait_ge(c2sem, 16)  # H tables in the 2nd const DMA
                    first_is1[0] = False
                cs = range(4 * h, 4 * h + 4)
                tensor.wait_ge(vsem, VOP[(1, s, h, 1)])  # P2h ready
                for c in cs:
                    # even channel opens its bank; odd writes the other half
                    mm(
                        rg[:, 256 * (c % 4) : 256 * (c % 4) + 256],
                        CRb[b][:, 128 * c : 128 * c + 128],
                        ca[:, _HA : _HA + 256],
                        start=(c % 2 == 0),
                        stop=False,
                    )
                tensor.wait_ge(vsem, VOP[(1, s, h, 2)])  # Q2nh ready
                for c in cs:
                    mm(
                        rg[:, 256 * (c % 4) : 256 * (c % 4) + 256],
                        CIb[b][:, 128 * c : 128 * c + 128],
                        ca[:, _HB : _HB + 256],
                        start=False,
                        stop=False,
                    )
                tensor.wait_ge(vsem, VOP[(1, s, h, 4)])  # R2h ready
                for c in cs:
                    i = mm(
                        rg[:, 256 * (c % 4) : 256 * (c % 4) + 256],
                        M2[b][:, 128 * c : 128 * c + 128],
                        ca[:, _HR : _HR + 256],
                        start=False,
                        stop=(c % 2 == 1),
                    )
                    if c % 4 == 3:
                        i.then_inc(psem, 1)

            def phase_is2(s, h):
                b = s % NB
                rg = psR[s % 2][:, 1024 * h : 1024 * h + 1024]
                cs = range(4 * h, 4 * h + 4)
                tensor.wait_ge(vsem, VOP[(2, s, h, 1)])  # P3bh ready
                for c in cs:
                    mm(
                        rg[:, 128 * (c % 4) : 128 * (c % 4) + 128],
                        P3b[b][:, 128 * c : 128 * c + 128],
                        ca[:, _KP : _KP + 128],
                        start=(c % 4 == 0),
                        stop=False,
                    )
                tensor.wait_ge(vsem, VOP[(2, s, h, 2)])  # Q3nbh ready
                for c in cs:
                    mm(
                        rg[:, 128 * (c % 4) : 128 * (c % 4) + 128],
                        Q3nb[b][:, 128 * c : 128 * c + 128],
                        ca[:, _KQ : _KQ + 128],
                        start=False,
                        stop=False,
                    )
                tensor.wait_ge(vsem, VOP[(2, s, h, 4)])  # R3bh ready
                for c in cs:
                    i = mm(
                        rg[:, 128 * (c % 4) : 128 * (c % 4) + 128],
                        R3b[b][:, 128 * c : 128 * c + 128],
                        ca[:, _KR : _KR + 128],
                        start=False,
                        stop=(c % 4 == 3),
                    )
                    if c % 4 == 3:
                        i.then_inc(psem, 1)

            phase_fns = [phase_s1, phase_s2, phase_is1, phase_is2]
            for ph, s, h in pe_order:
                phase_fns[ph](s, h)

        @block.scalar
        def _(scalar):
            for ph, s, h in act_order:
                b = s % NB
                ps = psR[s % 2]
                scalar.wait_ge(psem, PSEM[(ph, s, h)])
                if ph == 3:
                    if s >= NB and h == 0:
                        scalar.wait_ge(osem[s - NB], 32)
                    nc.scalar.copy(
                        ob[b][:, 512 * h : 512 * h + 512],
                        ps[:, 1024 * h : 1024 * h + 512],
                    ).then_inc(ssem, 1)
                else:
                    dst = [Yf, Ff, Sf][ph][b]
                    o = slice(1024 * h, 1024 * h + 1024)
                    nc.scalar.copy(dst[:, o], ps[:, o]).then_inc(ssem, 1)

    for t in reversed(ctx_list):
        t.__exit__(None, None, None)

    return nc


def _get_module():
    if "nc" not in _MODULE_CACHE:
        _MODULE_CACHE["nc"] = _build_module()
    return _MODULE_CACHE["nc"]


# ---------------------------------------------------------------------------
# host side
# ---------------------------------------------------------------------------


def _host_tables(rpm):
    """Per-sample chirp tables + Fb planes (un-replicated)."""
    pad = np.floor((RES * 60.0 / rpm.astype(np.float64) - TS) * SF).astype(np.int64)
    n_arr = L + pad
    t = np.arange(L, dtype=np.int64)
    m = np.arange(M, dtype=np.int64)
    mm = np.minimum(m, M - m)

    ach = np.empty((B, 2, 64, 128), np.float16)
    fbp = np.empty((B, 3, 128, 128), np.float16)
    for b in range(B):
        n = int(n_arr[b])
        two_n = 2 * n
        ph = np.pi * ((t * t) % two_n) / n
        ach[b, 0] = np.cos(ph).astype(np.float16).reshape(64, 128)
        ach[b, 1] = (-np.sin(ph)).astype(np.float16).reshape(64, 128)
        phb = np.pi * ((mm * mm) % two_n) / n
        Fb = np.fft.fft(np.exp(1j * phb)).reshape(128, 128) * FBSCALE
        fbp[b, 0] = Fb.real.astype(np.float16)
        fbp[b, 1] = (-Fb.imag).astype(np.float16)
        fbp[b, 2] = (Fb.real + Fb.imag).astype(np.float16)
    return ach, fbp


LAST_EXEC_WALL_NS = [None]


def kernel(inputs, rpm):
    inputs = np.ascontiguousarray(inputs, dtype=np.float32)  # [B, L, C]
    rpm = np.ascontiguousarray(rpm, dtype=np.float32)

    ca, cb = _consts()
    ach, fbp = _host_tables(rpm)
    xt = np.ascontiguousarray(inputs.transpose(0, 2, 1)).astype(np.float16)

    nc = _get_module()
    in_maps = []
    for g in range(NCORES):
        s0 = g * SPC
        in_maps.append(
            {
                "xt": xt[s0 : s0 + SPC],
                "ach": ach[s0 : s0 + SPC],
                "fbd": fbp[s0 : s0 + SPC],
                "cad": ca,
                "cbd": cb,
            }
        )

    import time

    from concourse.bass_utils import run_bass_kernel_spmd

    t0 = time.perf_counter_ns()
    res = run_bass_kernel_spmd(nc, in_maps, list(range(NCORES)))
    LAST_EXEC_WALL_NS[0] = time.perf_counter_ns() - t0

    out = np.empty((B, L, C), np.float32)
    for g in range(NCORES):
        planes = np.asarray(res.results[g]["outr"], np.float32)  # [SPC, 128, 1024]
        arr = planes.reshape(SPC, 128, C, 2, 64)  # [s, m1, c, re|im, m2]
        mag = np.hypot(arr[:, :, :, 0, :], arr[:, :, :, 1, :])  # [s, m1, c, m2]
        # conv index k = m1 + 128*m2  ->  out[s, k, c]
        out[g * SPC : (g + 1) * SPC] = (
            mag.transpose(0, 3, 1, 2).reshape(SPC, L, C)
        )
    return out


# revision 25
# speedup vs baseline: 1.3889x; 1.0732x over previous
"""EngineOrderFFT (Bluestein chirp-Z, fixed M=16384) Trainium2 kernel, V2.

Strategy
--------
Pure data parallelism: batch dim B=64 split across 8 NeuronCores
(8 samples/core). Each sample's variable-length DFT (length n_b) is a
Bluestein transform with fixed FFT size M=16384 = 128*128; each
16384-point (i)FFT is a two-stage Cooley-Tukey factorization executed as
fp16 matmuls on the tensor engine.

V2 changes vs V1:
  * is2 (inverse stage 2) runs in data-as-lhsT form: per channel the
    twiddled planes P3/Q3n/R3 [k2,m1] are the stationary operand and the
    Karatsuba combine tables K_P/K_Q/K_R [k2, m2(re|im)] are the moving
    operand -> 3072 output columns/sample instead of 6144, and the output
    lands as [m1, ch*(re64|im64)] = [128, 1024] (half the PSUM, cheaper
    evacuation).
  * All chirp/twiddle/Fb tables are stored un-replicated; the elementwise
    products broadcast them across the 8 channel pages with 0-stride APs
    (same DVE cost, 8x less DMA + SBUF).

Engine split per sample (8 channels batched in every instruction):
  sync   in/out DMAs
  gpsimd a-planes (x*chirp)
  PE     4 matmul stages (s1 16mm/4096col, s2 24mm/6144, is1 24mm/6144,
         is2 24mm/3072)
  ACT    PSUM -> fp16 SBUF evacuation after each stage
  DVE    twiddle/pointwise Karatsuba product planes (fp16 2x mode)

|conv[k]| is computed on the host from the shipped re/im planes.
"""
import numpy as np

SF, RES, TS = 8192, 40, 1
B, L, C = 64, 8192, 8
M = 16384
NCORES = 8
SPC = B // NCORES  # samples per core

FBSCALE = 1.0 / 32.0
HSCALE = 1.0 / 16.0
KSCALE = 1.0 / 32.0  # HSCALE*KSCALE = (1/M) * (1/FBSCALE)

# ---------------------------------------------------------------------------
# constant tables (input-independent)
# ---------------------------------------------------------------------------


def _f16(x):
    return np.ascontiguousarray(x, dtype=np.float16)


def _build_const_tables():
    j = np.arange(128)
    D = np.exp(-2j * np.pi * np.outer(j, j) / 128.0)  # symmetric
    Dc = np.conj(D)
    Wt = np.exp(-2j * np.pi * np.outer(j, j) / M)  # fwd twiddle [n1,k2]
    W2 = np.conj(Wt)  # inv twiddle
    Dr, Di = D.real, D.imag
    Hr, Hi = (Dc * HSCALE).real, (Dc * HSCALE).imag
    Kr, Ki = (Dc * KSCALE).real[:, :64], (Dc * KSCALE).imag[:, :64]

    cols = []
    # chunk 1 (cols 0:1024): W twiddle (L1), F tables (s2)
    cols += [Wt.real, -Wt.imag, Wt.real + Wt.imag]  # _WR,_WNI,_WS [128,128]
    cols += [Dr + Di, Dr - Di, Di - Dr, -Di, Dr]  # F1,F2,F2n,F3,F4 [128,128]
    # chunk 2 (cols 1024:2560): H (is1), W2 (L3), K (is2)
    cols += [np.concatenate([Hr + Hi, Hi - Hr], 1)]  # H_P [128,256]
    cols += [np.concatenate([Hr - Hi, Hi + Hr], 1)]  # H_Q
    cols += [np.concatenate([-Hi, Hr], 1)]  # H_R
    cols += [W2.real, -W2.imag, W2.real + W2.imag]  # _W2R,_W2NI,_W2S
    cols += [np.concatenate([Kr + Ki, Ki - Kr], 1)]  # K_P [128,128]
    cols += [np.concatenate([Kr - Ki, Ki + Kr], 1)]  # K_Q
    cols += [np.concatenate([-Ki, Kr], 1)]  # K_R
    ca = _f16(np.concatenate(cols, axis=1))

    cb = _f16(
        np.concatenate([Dr[:64], Di[:64], -Di[:64], Dr[:64]], axis=1)
    )  # [64, 512] = Dtab1|Dtab2
    return ca, cb


# column offsets in ca
_WR, _WNI, _WS = 0, 128, 256
_F = [384, 512, 640, 768, 896]  # F1,F2,F2n,F3,F4
_HA, _HB, _HR = 1024, 1280, 1536
_W2R, _W2NI, _W2S = 1792, 1920, 2048
_KP, _KQ, _KR = 2176, 2304, 2432
CA1_COLS = 1024
CA_COLS = 2560

_CONST_CACHE = {}


def _consts():
    if "ca" not in _CONST_CACHE:
        ca, cb = _build_const_tables()
        assert ca.shape[1] == CA_COLS, ca.shape
        _CONST_CACHE["ca"] = ca
        _CONST_CACHE["cb"] = cb
    return _CONST_CACHE["ca"], _CONST_CACHE["cb"]


# ---------------------------------------------------------------------------
# device module
# ---------------------------------------------------------------------------

_MODULE_CACHE = {}


def _build_module():
    import concourse.bass as bass
    from concourse import mybir

    dt = mybir.dt
    NB = 2  # per-sample buffer depth

    nc = bass.Bass("TRN2", target_bir_lowering=False, debug=False)

    # packed input: [64, 0:1024] = x as [n2, c, n1], [64, 1024:1280] = chirp
    # (cos | -sin) as [n2, n1]
    xad = nc.dram_tensor("xad", [SPC, 64, 1280], dt.float16, kind="ExternalInput").ap()
    # Fb planes: [128, 0:512] = (Fbr,Fbr,-Fbi,-Fbi), [128, 512:640] = Fbr+Fbi
    fbd = nc.dram_tensor("fbd", [SPC, 128, 640], dt.float16, kind="ExternalInput").ap()
    cad = nc.dram_tensor("cad", [128, CA_COLS], dt.float16, kind="ExternalInput").ap()
    cbd = nc.dram_tensor("cbd", [64, 512], dt.float16, kind="ExternalInput").ap()
    # out: [SPC, 128(m1), 8ch * (re64|im64)(m2)] ; conv[m1+128*m2]
    outr = nc.dram_tensor("outr", [SPC, 128, 1024], dt.float16, kind="ExternalOutput").ap()

    ctx_list = []

    def sb(name, shape, dtype=None):
        t = nc.sbuf_tensor(name, shape, dtype or mybir.dt.float16)
        ap = t.__enter__()
        ctx_list.append(t)
        return ap

    def psum(name, shape):
        t = nc.psum_tensor(name, shape, mybir.dt.float32)
        ap = t.__enter__()
        ctx_list.append(t)
        return ap

    ca = sb("ca", [128, CA_COLS])
    cb = sb("cb", [64, 512])
    ND = 6  # input/a-plane buffer depth (Pool must run well ahead)
    xa_t = [sb(f"xa{i}", [64, 1280]) for i in range(ND)]
    fbR = [sb(f"fbR{i}", [128, 640]) for i in range(ND)]
    A_t = [sb(f"A{i}", [64, 2048]) for i in range(ND)]
    Yf = [sb(f"Yf{i}", [128, 2048]) for i in range(NB)]
    PQ1 = [sb(f"PQ1_{i}", [128, 2048]) for i in range(NB)]
    Rb = [sb(f"Rb{i}", [128, 1024]) for i in range(NB)]
    Ff = [sb(f"Ff{i}", [128, 2048]) for i in range(NB)]
    CRI = [sb(f"CRI{i}", [128, 2048]) for i in range(NB)]
    Sf = [sb(f"Sf{i}", [128, 2048]) for i in range(NB)]
    PQ3 = [sb(f"PQ3_{i}", [128, 2048]) for i in range(NB)]
    R3b = [sb(f"R3b{i}", [128, 1024]) for i in range(NB)]
    ob = [sb(f"ob{i}", [128, 1024]) for i in range(NB)]
    S1 = [sb(f"S1_{i}", [128, 1024]) for i in range(NB)]
    S3 = [sb(f"S3_{i}", [128, 1024]) for i in range(NB)]
    M1 = [sb(f"M1_{i}", [128, 1024]) for i in range(NB)]
    M2 = [sb(f"M2_{i}", [128, 1024]) for i in range(NB)]

    # two 4-bank psum regions; samples alternate regions by parity
    psR = [psum("psR0", [128, 2048]), psum("psR1", [128, 2048])]

    csem = nc.alloc_semaphore("csem")
    c2sem = nc.alloc_semaphore("c2sem")
    cbsem = nc.alloc_semaphore("cbsem")
    smp = [nc.alloc_semaphore(f"smp{i}") for i in range(SPC)]
    osem = [nc.alloc_semaphore(f"osem{i}") for i in range(SPC)]
    fsem = [nc.alloc_semaphore(f"fsem{i}") for i in range(SPC)]
    vsem = nc.alloc_semaphore("vsem")
    psem = nc.alloc_semaphore("psem")
    ssem = nc.alloc_semaphore("ssem")
    gsem = nc.alloc_semaphore("gsem")

    # ---- emission orders (pair-interleaved, half-split) and sem tables ----
    # Every evac / product layer / PE phase is split into channel halves
    # h=0 (ch 0-3) and h=1 (ch 4-7) so the evac->product->matmul chain per
    # half is ~1.7us instead of ~3us and PE stays fed.
    pairs = [(2 * p, 2 * p + 1) for p in range(SPC // 2)]

    pe_order = []   # (phase, s, h): half-sample units
    act_order = []  # (evac, s, h)
    dve_order = []  # (group, s), group in 0..2 (L1, CL, L3); halves inside
    gp_order = []   # (kind, s), kind 0=a-planes
    # Software pipeline: pair p's is2 window (short, product-gated units) is
    # filled with pair p+1's s1 units, and ACT mirrors that order so the ob
    # evacs (which free s1 regions) and the next Yf evacs (which feed the L1
    # chain) land just-in-time.
    for p, (sa, sb_) in enumerate(pairs):
        if p == 0:
            pe_order += [(0, sa, 0), (0, sa, 1), (0, sb_, 0), (0, sb_, 1)]
            act_order += [(0, sa, 0), (0, sa, 1), (0, sb_, 0), (0, sb_, 1)]
        for ph in (1, 2):
            pe_order += [(ph, sa, 0), (ph, sa, 1), (ph, sb_, 0), (ph, sb_, 1)]
            act_order += [(ph, sa, 0), (ph, sa, 1), (ph, sb_, 0), (ph, sb_, 1)]
        if p + 1 < len(pairs):
            na, nb_ = pairs[p + 1]
            pe_order += [
                (3, sa, 0), (3, sa, 1), (0, na, 0), (3, sb_, 0),
                (0, na, 1), (3, sb_, 1), (0, nb_, 0), (0, nb_, 1),
            ]
            act_order += [
                (3, sa, 0), (3, sa, 1), (0, na, 0), (3, sb_, 0),
                (0, na, 1), (3, sb_, 1), (0, nb_, 0), (0, nb_, 1),
            ]
        else:
            pe_order += [(3, sa, 0), (3, sa, 1), (3, sb_, 0), (3, sb_, 1)]
            act_order += [(3, sa, 0), (3, sa, 1), (3, sb_, 0), (3, sb_, 1)]
        for g in range(3):
            dve_order += [(g, sa), (g, sb_)]
        gp_order += [(0, sa), (0, sb_)]
    gp_order = [e for e in gp_order if e[1] >= 2]

    # PE increments psem once per half-phase, in emission order.
    PSEM = {}
    for i, key in enumerate(pe_order):
        PSEM[key] = i + 1
    SSEM = {}
    for i, key in enumerate(act_order):
        SSEM[key] = i + 1
    GSEM = {}
    g = 0
    for kind, s in gp_order:
        g += 2
        GSEM[(kind, s)] = g
    # DVE op positions per (group, half): PQ(+1), S(+2), R(+3)
    VSEM = {}
    VOP = {}
    v = 4  # 4 startup a-plane ops on DVE (samples 0,1)
    for grp, s in dve_order:
        for h in range(2):
            for k in range(1, 4):
                VOP[(grp, s, h, k)] = v + 3 * h + k
        v += 6
        VSEM[(grp, s)] = v

    AluOp = mybir.AluOpType

    def bcast8(tab):
        """[P,128] table -> [P, 8, 128] zero-stride channel broadcast."""
        p = tab.shape[0]
        return tab.rearrange("p (o u) -> p o u", o=1).broadcast_to((p, 8, 128))

    with nc.Block() as block:

        @block.sync
        def _(sync):
            def emit_in(s):
                b = s % NB
                b4 = s % ND
                if s >= ND:
                    if s - ND < 2:
                        sync.wait_ge(vsem, 2 * (s - ND + 1))
                    else:
                        sync.wait_ge(gsem, GSEM[(0, s - ND)])
                if s >= ND:
                    sync.wait_ge(vsem, VSEM[(1, s - ND)])
                sync.dma_start(xa_t[b4][:], xad[s]).then_inc(smp[s], 16)
                sync.dma_start(fbR[s % ND][:], fbd[s]).then_inc(fsem[s], 16)

            def emit_out(s):
                b = s % NB
                for jj in range(2):
                    sync.wait_ge(ssem, SSEM[(3, s, jj)])
                    sync.dma_start(
                        outr[s][:, 512 * jj : 512 * jj + 512],
                        ob[b][:, 512 * jj : 512 * jj + 512],
                    ).then_inc(osem[s], 16)

            # startup: xa0 first so the a-planes (and s1) start ASAP;
            # const tables go down the scalar engine's DMA queue in parallel
            sync.dma_start(xa_t[0][:], xad[0]).then_inc(smp[0], 16)
            sync.dma_start(fbR[0][:], fbd[0]).then_inc(fsem[0], 16)
            emit_in(1)
            for s_ in range(2, ND):
                emit_in(s_)
            for s in range(SPC):
                if s + ND < SPC:
                    emit_in(s + ND)
                emit_out(s)

        @block.gpsimd
        def _(gp):
            for kind, s in gp_order:
                b4 = s % ND
                gp.wait_ge(smp[s], 16)
                if s >= ND:
                    gp.wait_ge(psem, PSEM[(0, s - ND, 1)])  # A_t[b4] free
                xv = xa_t[b4][:, 0:1024].rearrange("p (c n) -> p c n", c=C)
                nc.gpsimd.tensor_tensor(
                    A_t[b4][:, 0:1024].rearrange("p (c n) -> p c n", c=C),
                    xv,
                    bcast8(xa_t[b4][:, 1024:1152]),
                    AluOp.mult,
                ).then_inc(gsem, 1)
                nc.gpsimd.tensor_tensor(
                    A_t[b4][:, 1024:2048].rearrange("p (c n) -> p c n", c=C),
                    xv,
                    bcast8(xa_t[b4][:, 1152:1280]),
                    AluOp.mult,
                ).then_inc(gsem, 1)

        @block.vector
        def _(vector):
            def chpages(ap):
                v_ = ap.rearrange("p (c u) -> p c u", c=C)
                return v_[:, :, 0:128], v_[:, :, 128:256]

            def prpages(ap):
                # s2 output layout: 4 q-blocks of (re 2ch*128 | im 2ch*128)
                v_ = ap.rearrange("p (q r u) -> p q r u", q=4, r=2)
                return v_[:, :, 0, :], v_[:, :, 1, :]  # [128, 4, 256] each

            def flat8(ap):
                return ap.rearrange("p (c u) -> p c u", c=C)

            def flat4(ap):
                return ap.rearrange("p (q u) -> p q u", q=4)

            def bcast4x2(tab):
                # [128,128] -> [128, 4, 2, 128] for the q-block × 2ch layout
                return tab.rearrange("p (o q u) -> p o q u", o=1, q=1).broadcast_to(
                    (128, 4, 2, 128)
                )

            for s0 in (0, 1):
                vector.wait_ge(smp[s0], 16)
                xv = xa_t[s0][:, 0:1024].rearrange("p (c n) -> p c n", c=C)
                nc.vector.tensor_tensor(
                    A_t[s0][:, 0:1024].rearrange("p (c n) -> p c n", c=C),
                    xv,
                    bcast8(xa_t[s0][:, 1024:1152]),
                    AluOp.mult,
                ).then_inc(vsem, 1)
                nc.vector.tensor_tensor(
                    A_t[s0][:, 1024:2048].rearrange("p (c n) -> p c n", c=C),
                    xv,
                    bcast8(xa_t[s0][:, 1152:1280]),
                    AluOp.mult,
                ).then_inc(vsem, 1)
            def bc_h(tab, n, w):
                # [128,w] table -> [128, n, w] zero-stride broadcast
                return tab.rearrange("p (o u) -> p o u", o=1).broadcast_to(
                    (128, n, w)
                )

            first_dve = [True]
            for grp, s in dve_order:
                if first_dve[0]:
                    vector.wait_ge(csem, 16)
                    first_dve[0] = False
                    first_l3 = [True]
                b = s % NB
                if grp == 0:
                    # L1 (fwd twiddle, Karatsuba planes) from Yf, by halves
                    if s >= NB:
                        vector.wait_ge(psem, PSEM[(1, s - NB, 1)])  # bufs free
                    yv = Yf[b][:].rearrange("p (c u) -> p c u", c=C)
                    pv = PQ1[b][:].rearrange("p (c u) -> p c u", c=C)
                    for h in range(2):
                        vector.wait_ge(ssem, SSEM[(0, s, h)])
                        c4 = slice(4 * h, 4 * h + 4)
                        nc.vector.tensor_tensor(
                            pv[:, c4, :], yv[:, c4, :],
                            bc_h(ca[:, 0:256], 4, 256), AluOp.mult,
                        ).then_inc(vsem, 1)  # [P|Qn] per channel
                        nc.vector.tensor_tensor(
                            flat8(S1[b][:])[:, c4, :],
                            yv[:, c4, 0:128], yv[:, c4, 128:256], AluOp.add,
                        ).then_inc(vsem, 1)
                        vector.wait_ge(vsem, VOP[(grp, s, h, 2)])  # S1h drained
                        nc.vector.tensor_tensor(
                            flat8(Rb[b][:])[:, c4, :],
                            flat8(S1[b][:])[:, c4, :],
                            bc_h(ca[:, _WS : _WS + 128], 4, 128), AluOp.mult,
                        ).then_inc(vsem, 1)
                elif grp == 1:
                    # C-layer (Fa o Fb, Karatsuba planes) from Ff, by halves
                    vector.wait_ge(fsem[s], 16)
                    if s >= NB:
                        vector.wait_ge(psem, PSEM[(2, s - NB, 1)])  # bufs free
                    fv = Ff[b][:].rearrange("p (q u) -> p q u", q=4)
                    cv = CRI[b][:].rearrange("p (q u) -> p q u", q=4)
                    for h in range(2):
                        vector.wait_ge(ssem, SSEM[(1, s, h)])
                        q2 = slice(2 * h, 2 * h + 2)
                        nc.vector.tensor_tensor(
                            cv[:, q2, :], fv[:, q2, :],
                            bc_h(fbR[s % ND][:, 0:512], 2, 512), AluOp.mult,
                        ).then_inc(vsem, 1)  # [CR 2ch | CI 2ch] per q
                        nc.vector.tensor_tensor(
                            flat4(M1[b][:])[:, q2, :],
                            fv[:, q2, 0:256], fv[:, q2, 256:512], AluOp.add,
                        ).then_inc(vsem, 1)
                        vector.wait_ge(vsem, VOP[(grp, s, h, 2)])  # M1h drained
                        nc.vector.tensor_tensor(
                            flat4(M2[b][:])[:, q2, :].rearrange(
                                "p q (c u) -> p q c u", c=2
                            ),
                            flat4(M1[b][:])[:, q2, :].rearrange(
                                "p q (c u) -> p q c u", c=2
                            ),
                            fbR[s % ND][:, 512:640].rearrange(
                                "p (o q u) -> p o q u", o=1, q=1
                            ).broadcast_to((128, 2, 2, 128)),
                            AluOp.mult,
                        ).then_inc(vsem, 1)
                else:
                    # L3 (inv twiddle, Karatsuba planes) from Sf, by halves
                    if first_l3[0]:
                        vector.wait_ge(c2sem, 16)
                        first_l3[0] = False
                    if s >= NB:
                        vector.wait_ge(psem, PSEM[(3, s - NB, 1)])  # bufs free
                    sv = Sf[b][:].rearrange("p (c u) -> p c u", c=C)
                    p3v = PQ3[b][:].rearrange("p (c u) -> p c u", c=C)
                    for h in range(2):
                        vector.wait_ge(ssem, SSEM[(2, s, h)])
                        c4 = slice(4 * h, 4 * h + 4)
                        nc.vector.tensor_tensor(
                            p3v[:, c4, :], sv[:, c4, :],
                            bc_h(ca[:, _W2R : _W2R + 256], 4, 256), AluOp.mult,
                        ).then_inc(vsem, 1)
                        nc.vector.tensor_tensor(
                            flat8(S3[b][:])[:, c4, :],
                            sv[:, c4, 0:128], sv[:, c4, 128:256], AluOp.add,
                        ).then_inc(vsem, 1)
                        vector.wait_ge(vsem, VOP[(grp, s, h, 2)])  # S3h drained
                        nc.vector.tensor_tensor(
                            flat8(R3b[b][:])[:, c4, :],
                            flat8(S3[b][:])[:, c4, :],
                            bc_h(ca[:, _W2S : _W2S + 128], 4, 128), AluOp.mult,
                        ).then_inc(vsem, 1)

        @block.tensor
        def _(tensor):
            mm = nc.tensor.matmul
            first_pe = [True]
            first_is1 = [True]

            def phase_s1(s, h):
                rg = psR[s % 2][:, 1024 * h : 1024 * h + 1024]
                if first_pe[0]:
                    tensor.wait_ge(cbsem, 16)  # cb loaded
                    first_pe[0] = False
                if h == 0:
                    if s < 2:
                        tensor.wait_ge(vsem, 2 * (s + 1))  # startup a-planes
                    else:
                        tensor.wait_ge(gsem, GSEM[(0, s)])
                if s >= NB:
                    # region free once ob evac half h of s-NB done
                    tensor.wait_ge(ssem, SSEM[(3, s - NB, h)])
                b4 = s % ND
                for c in range(4 * h, 4 * h + 4):
                    o = rg[:, 256 * (c - 4 * h) : 256 * (c - 4 * h) + 256]
                    mm(
                        o,
                        A_t[b4][:, 128 * c : 128 * c + 128],
                        cb[:, 0:256],
                        start=True,
                        stop=False,
                    )
                    i = mm(
                        o,
                        A_t[b4][:, 1024 + 128 * c : 1024 + 128 * c + 128],
                        cb[:, 256:512],
                        start=False,
                        stop=True,
                    )
                    if c % 4 == 3:
                        i.then_inc(psem, 1)

            def phase_s2(s, h):
                b = s % NB
                rg = psR[s % 2][:, 1024 * h : 1024 * h + 1024]
                if h == 0:
                    tensor.wait_ge(csem, 16)  # ca chunk1 loaded
                def pq1(q, r):
                    # plane r (0=P, 1=Qn) of channels 2q, 2q+1 from PQ1
                    return PQ1[b][:, 512 * q : 512 * q + 512].rearrange(
                        "p (c r u) -> p c r u", c=2, r=2
                    )[:, :, r, :]

                srcs = [
                    (0, _F[0], 0, True, False, 1),
                    (1, _F[0], 256, False, False, None),
                    (1, _F[1], 0, False, False, None),
                    (0, _F[2], 256, False, False, None),
                    (2, _F[3], 0, False, False, 3),
                    (2, _F[4], 256, False, True, None),
                ]
                for wi, (pr, fofs, oofs, st, sp, wk) in enumerate(srcs):
                    if wk is not None:
                        tensor.wait_ge(vsem, VOP[(0, s, h, wk)])
                    for ql in range(2):
                        q = 2 * h + ql
                        rhs = (
                            Rb[b][:, 256 * q : 256 * q + 256]
                            if pr == 2
                            else pq1(q, pr)
                        )
                        i = mm(
                            rg[:, 512 * ql + oofs : 512 * ql + oofs + 256],
                            ca[:, fofs : fofs + 128],
                            rhs,
                            start=st,
                            stop=sp,
                        )
                        if wi == 5 and ql == 1:
                            i.then_inc(psem, 1)

            def phase_is1(s, h):
                b = s % NB
                rg = psR[s % 2][:, 1024 * h : 1024 * h + 1024]
                if first_is1[0]:
                    tensor.wait_ge(c2sem, 16)  # H tables in the 2nd const DMA
                    first_is1[0] = False
                cs = range(4 * h, 4 * h + 4)
                tensor.wait_ge(vsem, VOP[(1, s, h, 1)])  # [CR|CI]h ready
                for c in cs:
                    # even channel opens its bank; odd writes the other half
                    mm(
                        rg[:, 256 * (c % 4) : 256 * (c % 4) + 256],
                        CRI[b][
                            :,
                            512 * (c // 2)
                            + 128 * (c % 2) : 512 * (c // 2)
                            + 128 * (c % 2)
                            + 128,
                        ],
                        ca[:, _HA : _HA + 256],
                        start=(c % 2 == 0),
                        stop=False,
                    )
                for c in cs:
                    mm(
                        rg[:, 256 * (c % 4) : 256 * (c % 4) + 256],
                        CRI[b][
                            :,
                            512 * (c // 2)
                            + 256
                            + 128 * (c % 2) : 512 * (c // 2)
                            + 256
                            + 128 * (c % 2)
                            + 128,
                        ],
                        ca[:, _HB : _HB + 256],
                        start=False,
                        stop=False,
                    )
                tensor.wait_ge(vsem, VOP[(1, s, h, 3)])  # M2h ready
                for c in cs:
                    i = mm(
                        rg[:, 256 * (c % 4) : 256 * (c % 4) + 256],
                        M2[b][:, 128 * c : 128 * c + 128],
                        ca[:, _HR : _HR + 256],
                        start=False,
                        stop=(c % 2 == 1),
                    )
                    if c % 4 == 3:
                        i.then_inc(psem, 1)

            def phase_is2(s, h):
                b = s % NB
                rg = psR[s % 2][:, 1024 * h : 1024 * h + 1024]
                cs = range(4 * h, 4 * h + 4)
                tensor.wait_ge(vsem, VOP[(2, s, h, 1)])  # [P3|Q3n]h ready
                for c in cs:
                    mm(
                        rg[:, 128 * (c % 4) : 128 * (c % 4) + 128],
                        PQ3[b][:, 256 * c : 256 * c + 128],
                        ca[:, _KP : _KP + 128],
                        start=(c % 4 == 0),
                        stop=False,
                    )
                for c in cs:
                    mm(
                        rg[:, 128 * (c % 4) : 128 * (c % 4) + 128],
                        PQ3[b][:, 256 * c + 128 : 256 * c + 256],
                        ca[:, _KQ : _KQ + 128],
                        start=False,
                        stop=False,
                    )
                tensor.wait_ge(vsem, VOP[(2, s, h, 3)])  # R3bh ready
                for c in cs:
                    i = mm(
                        rg[:, 128 * (c % 4) : 128 * (c % 4) + 128],
                        R3b[b][:, 128 * c : 128 * c + 128],
                        ca[:, _KR : _KR + 128],
                        start=False,
                        stop=(c % 4 == 3),
                    )
                    if c % 4 == 3:
                        i.then_inc(psem, 1)

            phase_fns = [phase_s1, phase_s2, phase_is1, phase_is2]
            for ph, s, h in pe_order:
                phase_fns[ph](s, h)

        @block.scalar
        def _(scalar):
            nc.scalar.dma_start(cb[:], cbd[:]).then_inc(cbsem, 16)
            nc.scalar.dma_start(ca[:, 0:CA1_COLS], cad[:, 0:CA1_COLS]).then_inc(
                csem, 16
            )
            nc.scalar.dma_start(ca[:, CA1_COLS:], cad[:, CA1_COLS:]).then_inc(
                c2sem, 16
            )
            for ph, s, h in act_order:
                b = s % NB
                ps = psR[s % 2]
                scalar.wait_ge(psem, PSEM[(ph, s, h)])
                if ph == 3:
                    if s >= NB and h == 0:
                        scalar.wait_ge(osem[s - NB], 32)
                    nc.scalar.copy(
                        ob[b][:, 512 * h : 512 * h + 512],
                        ps[:, 1024 * h : 1024 * h + 512],
                    ).then_inc(ssem, 1)
                else:
                    dst = [Yf, Ff, Sf][ph][b]
                    o = slice(1024 * h, 1024 * h + 1024)
                    nc.scalar.copy(dst[:, o], ps[:, o]).then_inc(ssem, 1)

    for t in reversed(ctx_list):
        t.__exit__(None, None, None)

    return nc


def _get_module():
    if "nc" not in _MODULE_CACHE:
        _MODULE_CACHE["nc"] = _build_module()
    return _MODULE_CACHE["nc"]


# ---------------------------------------------------------------------------
# host side
# ---------------------------------------------------------------------------


def _host_tables(rpm):
    """Per-sample chirp tables + Fb planes (un-replicated)."""
    pad = np.floor((RES * 60.0 / rpm.astype(np.float64) - TS) * SF).astype(np.int64)
    n_arr = L + pad
    t = np.arange(L, dtype=np.int64)
    m = np.arange(M, dtype=np.int64)
    mm = np.minimum(m, M - m)

    ach = np.empty((B, 256), np.float16)   # per n2-row: [cos 128 | -sin 128]
    fbp = np.empty((B, 128, 640), np.float16)
    for b in range(B):
        n = int(n_arr[b])
        two_n = 2 * n
        ph = np.pi * ((t * t) % two_n) / n
        cosv = np.cos(ph).astype(np.float16).reshape(64, 128)
        nsin = (-np.sin(ph)).astype(np.float16).reshape(64, 128)
        ach[b] = 0  # unused filler; real packing happens in kernel()
        _ACH_COS[b] = cosv
        _ACH_NSIN[b] = nsin
        phb = np.pi * ((mm * mm) % two_n) / n
        Fb = np.fft.fft(np.exp(1j * phb)).reshape(128, 128) * FBSCALE
        fr = Fb.real.astype(np.float16)
        fni = (-Fb.imag).astype(np.float16)
        fs = (Fb.real + Fb.imag).astype(np.float16)
        fbp[b] = np.concatenate([fr, fr, fni, fni, fs], axis=1)
    return fbp


_ACH_COS = np.empty((B, 64, 128), np.float16)
_ACH_NSIN = np.empty((B, 64, 128), np.float16)


LAST_EXEC_WALL_NS = [None]


def kernel(inputs, rpm):
    inputs = np.ascontiguousarray(inputs, dtype=np.float32)  # [B, L, C]
    rpm = np.ascontiguousarray(rpm, dtype=np.float32)

    ca, cb = _consts()
    fbp = _host_tables(rpm)
    # pack per-sample [64, 1280]: x as [n2, c, n1] cols 0:1024, chirp cols
    # 1024:1280 = [cos | -sin]
    xa = np.empty((B, 64, 1280), np.float16)
    xv = inputs.reshape(B, 64, 128, C).transpose(0, 1, 3, 2)  # [B, n2, c, n1]
    xa[:, :, 0:1024] = xv.reshape(B, 64, 1024).astype(np.float16)
    xa[:, :, 1024:1152] = _ACH_COS
    xa[:, :, 1152:1280] = _ACH_NSIN

    nc = _get_module()
    in_maps = []
    for g in range(NCORES):
        s0 = g * SPC
        in_maps.append(
            {
                "xad": xa[s0 : s0 + SPC],
                "fbd": fbp[s0 : s0 + SPC],
                "cad": ca,
                "cbd": cb,
            }
        )

    import time

    from concourse.bass_utils import run_bass_kernel_spmd

    t0 = time.perf_counter_ns()
    res = run_bass_kernel_spmd(nc, in_maps, list(range(NCORES)))
    LAST_EXEC_WALL_NS[0] = time.perf_counter_ns() - t0

    out = np.empty((B, L, C), np.float32)
    for g in range(NCORES):
        planes = np.asarray(res.results[g]["outr"], np.float32)  # [SPC, 128, 1024]
        arr = planes.reshape(SPC, 128, C, 2, 64)  # [s, m1, c, re|im, m2]
        mag = np.hypot(arr[:, :, :, 0, :], arr[:, :, :, 1, :])  # [s, m1, c, m2]
        # conv index k = m1 + 128*m2  ->  out[s, k, c]
        out[g * SPC : (g + 1) * SPC] = (
            mag.transpose(0, 3, 1, 2).reshape(SPC, L, C)
        )
    return out
